# revision 1
# baseline (speedup 1.0000x reference)
"""DiT block with block-diffusion sparse attention on 8 Trainium2 NeuronCores.

v3 strategy:
  - adaLN modulation computed on HOST (tiny matvec); gamma scales folded into
    the QKV / MLP1 weights, shifts folded into bias rows.
  - LN1 never materializes h: QKV matmuls read host-pretransposed x^T
    directly; the per-token (-mu, sigma) correction enters the same PSUM as a
    rank-2 matmul with stationary (u, b) = ((W*gamma)@1, W@beta); the 1/sigma
    scale is folded into the RoPE cos/sin tables. Stats come from a
    token-major bn_stats pass.
  - Attention: 2 heads per core, scores in S^T orientation, both heads of an
    item share one PSUM tile / one exp / one mask op. Softmax normalization
    is DEFERRED: unnormalized o + denominator rows are evacuated per stream,
    reciprocals batched per half (keeps gpsimd/vector off the per-stream
    critical path).
  - PSUM is always evacuated through ScalarE (ACT) before VectorE touches the
    data - DVE reads from PSUM are ~4x slower than from SBUF.
  - One AllToAll converts head-sharded attention output to token-sharded.
  - attn_out, LN2, MLP token-sharded with full bf16 weights prefetched during
    the attention/A2A window. Output written feature-major; host transposes.
"""

import os
import numpy as np
import ml_dtypes

import concourse.bass as bass
import concourse.tile as tile
from concourse import bacc, mybir
from concourse.bass_utils import run_bass_kernel_spmd
from concourse.masks import make_identity

bf16 = ml_dtypes.bfloat16
fp8 = ml_dtypes.float8_e4m3
FP = mybir.dt.float32
BF = mybir.dt.bfloat16
F8 = mybir.dt.float8e4
AF = mybir.ActivationFunctionType
ALU = mybir.AluOpType
DR = mybir.MatmulPerfMode.DoubleRow
WQSCALE = 64.0
WAOSCALE = 64.0
W1SCALE = 64.0
W2SCALE = 128.0

NCORES = 8
S, N, D, H, HD, BS, COND = 2048, 1024, 1024, 16, 64, 16, 128
TOK = S // NCORES  # 256 tokens per core after A2A


def _attn_schedule():
    """Per q-chunk list of (ktile, col0, col1, mask) in S^T orientation."""
    sched = []
    for c in range(4):
        items = []
        if c < 2:  # noisy q chunk
            for j in range(4 * c + 4):  # clean k tiles, bq > bk
                js = j - 4 * c
                if js < 0:
                    items.append((8 + j, 0, 512, None))
                else:
                    items.append((8 + j, 128 * js, 512, "strict"))
            for s in range(4):  # own-block diagonal (noisy k)
                items.append((4 * c + s, 128 * s, 128 * s + 128, "diag"))
        else:  # clean q chunk, bq >= bk
            cq = c - 2
            for j in range(4 * cq + 4):
                js = j - 4 * cq
                if js < 0:
                    items.append((8 + j, 0, 512, None))
                else:
                    items.append((8 + j, 128 * js, 512, "incl"))
        assert items[0][1] == 0 and items[0][2] == 512
        sched.append(items)
    return sched


MASK_OFF = {"diag": 0, "strict": 256, "incl": 512}


def build_program(single=False, dbg=False):
    """single=True builds a 1-device variant (A2A replaced by a local DMA
    copy) for TimelineSim cost-model analysis. dbg=True adds a debug output
    with intermediate tensors."""
    nc = bacc.Bacc(
        "TRN2", target_bir_lowering=False, debug=False,
        enable_asserts=False, num_devices=1 if single else NCORES,
    )

    def din(name, shape, dt=BF):
        return nc.dram_tensor(name, shape, dt, kind="ExternalInput").ap()

    x_d = din("x", [S, D])                            # token-major (stats)
    xT_d = din("xT", [4, 128, 2, S], F8)              # DR pairs (j, p, i, t)
    xsT_d = din("xsliceT", [8, 128, TOK])             # residual slice (k, p, t)
    trig_d = din("trig", [128, 2 * S])                # cos2 | sin2(dest-signed)
    mask01_d = din("mask01", [128, 768])              # diag|strict|incl x2
    wqkv_d = din("wqkvT", [3, 128, 4, 2, 128], F8)    # (s, p, j, i, c) scaled
    ub_d = din("ubrow", [2, 384])                     # (u; b) per-core slice
    wao_d = din("waoT", [2, 64, 4, 8, 2, 128], F8)    # (g, p, mi, k, i, c)
    w1_d = din("w1T", [8, 128, 4, 4, 2, 128], F8)     # (g, p, mi, j, i, c)
    w2_d = din("w2T", [8, 128, 16, 2, 128], F8)       # (m, p, j, i, c)
    smallc_d = din("smallc", [128, 64], FP)           # gmsa|gmlp128|b1'|b2|gb2
    out_d = nc.dram_tensor("out", [8, 128, TOK], FP, kind="ExternalOutput").ap()
    dbg_d = (nc.dram_tensor("dbg", [8, 128, S], BF,
                            kind="ExternalOutput").ap() if dbg else None)

    sched = _attn_schedule()

    with tile.TileContext(nc) as tc:
        with tc.tile_pool(name="const", bufs=1) as const, \
             tc.tile_pool(name="dram", bufs=1, space="DRAM") as dram, \
             tc.tile_pool(name="qkvr", bufs=1) as qkvr, \
             tc.tile_pool(name="vaugp", bufs=1) as vaugp, \
             tc.tile_pool(name="x2p", bufs=1) as x2p, \
             tc.tile_pool(name="gp", bufs=1) as gp:

            # ---------------- early DMAs (stats x first, then xT) ------
            xstat = tc.alloc_tile_pool(name="xstat", bufs=2)
            xTp = tc.alloc_tile_pool(name="xTp", bufs=1)
            x_r = x_d.rearrange("(t p) d -> p t d", p=128)  # [128,16,D]
            xg_sb = []
            for g in range(4):
                xg = xstat.tile([128, 4, D], BF, tag="x", name=f"xg{g}")
                # gpsimd queue: keeps the stats path off the busy sync queue
                nc.gpsimd.dma_start(out=xg, in_=x_r[:, 4 * g:4 * g + 4, :])
                xg_sb.append(xg)
            xT_sb = [xTp.tile([128, 2, S], F8, name=f"xT{j}")
                     for j in range(4)]
            for j in range(4):
                nc.sync.dma_start(out=xT_sb[j], in_=xT_d[j])

            # ---------------- constants / small inputs ----------------
            trig_sb = const.tile([128, 2 * S], BF)
            nc.sync.dma_start(out=trig_sb, in_=trig_d)
            mask_sb = const.tile([128, 768], BF)
            nc.sync.dma_start(out=mask_sb, in_=mask01_d)
            smallc = const.tile([128, 64], FP)
            nc.sync.dma_start(out=smallc, in_=smallc_d)
            gmsa_sb = smallc[:, 0:8]
            gmlp_sb = smallc[:, 8:16]
            b1_sb = smallc[:, 16:48]
            b2_sb = smallc[:, 48:56]
            gb2_sb = smallc[:, 56:64]
            ub_sb = const.tile([2, 384], BF)
            nc.scalar.dma_start(out=ub_sb, in_=ub_d)
            ones_sb = const.tile([128, 1], BF)
            nc.vector.memset(ones_sb, 1.0)
            ones_row = const.tile([1, 128], BF)
            nc.vector.memset(ones_row, 1.0)
            eps128 = const.tile([128, 1], FP)
            nc.vector.memset(eps128, 1e-5)
            eps1 = const.tile([1, 1], FP)
            nc.vector.memset(eps1, 1e-5)
            ident_f = const.tile([128, 128], FP)
            make_identity(nc, ident_f)
            ident_b = const.tile([128, 128], BF)
            nc.vector.tensor_copy(out=ident_b, in_=ident_f)

            # residual slice (feature-major) straight from DRAM
            xsT = [x2p.tile([128, TOK], BF, name=f"xsT{j}") for j in range(8)]
            for fj in range(8):
                nc.sync.dma_start(out=xsT[fj], in_=xsT_d[fj])

            # qkv weights early on the scalar queue
            wqkvp = tc.alloc_tile_pool(name="wqkvp", bufs=1)
            wq_sb = [wqkvp.tile([128, 4 * 2 * 128], F8, name=f"wq{m}")
                     for m in range(3)]
            for m in range(3):
                nc.scalar.dma_start(
                    out=wq_sb[m].rearrange("p (j i c) -> p j i c",
                                           i=2, c=128),
                    in_=wqkv_d[m])
            wqr = [wq_sb[m].rearrange("p (j i c) -> p j i c", i=2, c=128)
                   for m in range(3)]

            # ---------------- phase 0: LN1 stats (token-major) ---------
            # per 128-token tile: bn stats -> (-mu, sd, rstd) columns of a
            # [128, 48] tile; one PE transpose + DRAM bounce turns them into
            # rows aligned with qT columns.
            stats_dr = dram.tile([3, S], BF)
            rows_sb = const.tile([2, S], BF)   # (negmu ; sd)
            rstd_row = const.tile([1, S], BF)
            with tc.tile_pool(name="statp", bufs=4) as statp, \
                 tc.tile_pool(name="st24", bufs=2) as st24p, \
                 tc.tile_pool(name="stps", bufs=2, space="PSUM") as stps:
                # pipelined per token-half: stats -> transpose -> bounce
                for half in range(2):
                    st24 = st24p.tile([128, 24], FP, tag="st24")
                    for g in range(2 * half, 2 * half + 2):
                        for sub in range(4):
                            ti = (4 * g + sub) % 8
                            x_sb = xg_sb[g][:, sub, :]
                            st = statp.tile([128, 2, 6], FP, tag="bst")
                            for sg in range(2):
                                nc.vector.bn_stats(
                                    out=st[:, sg, :],
                                    in_=x_sb[:, 512 * sg:512 * sg + 512])
                            mv = statp.tile([128, 2], FP, tag="mv")
                            nc.vector.bn_aggr(out=mv, in_=st)
                            nc.vector.tensor_scalar_mul(
                                st24[:, ti:ti + 1], mv[:, 0:1], -1.0)
                            nc.scalar.activation(
                                out=st24[:, 8 + ti:9 + ti], in_=mv[:, 1:2],
                                func=AF.Sqrt, bias=eps128, scale=1.0)
                            nc.vector.reciprocal(
                                out=st24[:, 16 + ti:17 + ti],
                                in_=st24[:, 8 + ti:9 + ti])
                    ps = stps.tile([24, 128], FP, tag="stt")
                    nc.tensor.transpose(ps, st24, ident_f)
                    st24b = st24p.tile([24, 128], BF, tag="st24b")
                    nc.vector.tensor_copy(out=st24b, in_=ps)
                    hs = slice(1024 * half, 1024 * half + 1024)
                    nc.gpsimd.dma_start(
                        out=stats_dr[:, hs].rearrange(
                            "v (t p) -> v t p", p=128),
                        in_=st24b)
                    nc.gpsimd.dma_start(out=rows_sb[:, hs],
                                        in_=stats_dr[0:2, hs])
                    nc.gpsimd.dma_start(out=rstd_row[:, hs],
                                        in_=stats_dr[2:3, hs])

            # rstd broadcast via PE rank-1 -> ACT evac -> fold into trig
            cosr = const.tile([128, S], BF)
            sinr = const.tile([128, S], BF)
            with tc.tile_pool(name="rbps", bufs=1, space="PSUM") as rbps, \
                 tc.tile_pool(name="rbt", bufs=1) as rbt:
                rstd_ps = rbps.tile([128, S], FP)
                for q in range(4):
                    nc.tensor.matmul(
                        rstd_ps[:, 512 * q:512 * q + 512], ones_row,
                        rstd_row[:, 512 * q:512 * q + 512],
                        start=True, stop=True, skip_group_check=True)
                rstd_bc = rbt.tile([128, S], BF)
                for q in range(2):
                    qs = slice(1024 * q, 1024 * q + 1024)
                    nc.scalar.copy(out=rstd_bc[:, qs], in_=rstd_ps[:, qs])
                    nc.vector.tensor_mul(cosr[:, qs], trig_sb[:, qs],
                                         rstd_bc[:, qs])
                    nc.vector.tensor_mul(
                        sinr[:, qs], trig_sb[:, S + 1024 * q:S + 1024 * q + 1024],
                        rstd_bc[:, qs])

            # ---------------- phase 1: QKV + RoPE ----------------------
            qT = qkvr.tile([128, S], BF)
            kT = qkvr.tile([128, S], BF)
            vT = qkvr.tile([128, S], BF)
            qkv_dst = [qT, kT, vT]

            with tc.tile_pool(name="ropep", bufs=3) as ropep, \
                 tc.tile_pool(name="qkvps", bufs=3, space="PSUM") as qkvps:
                for n in range(4):
                    nsl = slice(512 * n, 512 * n + 512)
                    for m in range(3):
                        ps = qkvps.tile([128, 512], FP, tag="qkvps")
                        for j in range(4):
                            nc.tensor.matmul(
                                ps, wqr[m][:, j],
                                xT_sb[j][:, :, nsl], perf_mode=DR,
                                start=(j == 0), stop=False)
                        # rank-2: + u (.) (-mu)  +  b (.) sd
                        nc.tensor.matmul(
                            ps, ub_sb[:, 128 * m:128 * m + 128],
                            rows_sb[0:2, nsl], start=False, stop=True,
                            skip_group_check=True)
                        # ACT evac, then rope on SBUF bf16:
                        # dst = pb*cosr + swap32(pb)*sinr  (sign in sinr)
                        pb = ropep.tile([128, 512], BF, tag="pb")
                        nc.scalar.copy(out=pb, in_=ps)
                        pbs = ropep.tile([128, 512], BF, tag="pbs")
                        for h in range(2):
                            r = 64 * h
                            nc.vector.tensor_copy(
                                out=pbs[r:r + 32, :], in_=pb[r + 32:r + 64, :])
                            nc.vector.tensor_copy(
                                out=pbs[r + 32:r + 64, :], in_=pb[r:r + 32, :])
                        t1 = ropep.tile([128, 512], BF, tag="t1")
                        nc.vector.tensor_mul(t1, pb, cosr[:, nsl])
                        t2 = ropep.tile([128, 512], BF, tag="t2")
                        nc.vector.tensor_mul(t2, pbs, sinr[:, nsl])
                        nc.vector.tensor_add(
                            qkv_dst[m][:, nsl], t1, t2)
            wqkvp.release()
            xTp.release()
            xstat.release()

            # ---------------- phase 2: V token-major (+ones col) -------
            vaug = [vaugp.tile([128, 130], BF, name=f"vaug{kt}")
                    for kt in range(16)]
            with tc.tile_pool(name="vtps", bufs=2, space="PSUM") as vtps:
                for kt in range(16):
                    ps = vtps.tile([128, 128], BF, tag="vt")
                    nc.tensor.transpose(
                        ps, vT[:, 128 * kt:128 * kt + 128], ident_b)
                    va = vaug[kt]
                    nc.vector.memset(va[:, 64:65], 1.0)
                    nc.vector.memset(va[:, 129:130], 1.0)
                    nc.scalar.copy(
                        out=va[:, 0:130].rearrange(
                            "p (h y) -> p h y", y=65)[:, :, 0:64],
                        in_=ps.rearrange("p (h d) -> p h d", d=64))

            # ---------------- weight prefetch (runs under attn + A2A) --
            waop = tc.alloc_tile_pool(name="waop", bufs=2)
            w1p = tc.alloc_tile_pool(name="w1p", bufs=8)
            w2p = tc.alloc_tile_pool(name="w2p", bufs=8)
            wao_sb = [waop.tile([64, 4 * 8 * 2 * 128], F8, tag="wao",
                                name=f"wao{g}")
                      for g in range(2)]
            for g in range(2):
                nc.scalar.dma_start(
                    out=wao_sb[g].rearrange("p (mi k i c) -> p mi k i c",
                                            k=8, i=2, c=128),
                    in_=wao_d[g])
            waor = [wao_sb[g].rearrange("p (mi k i c) -> p mi k i c",
                                        k=8, i=2, c=128) for g in range(2)]
            w1_sb = [w1p.tile([128, 4 * 4 * 2 * 128], F8, tag="w1",
                              name=f"w1_{g}")
                     for g in range(8)]
            for g in range(8):
                nc.scalar.dma_start(
                    out=w1_sb[g].rearrange("p (mi j i c) -> p mi j i c",
                                           j=4, i=2, c=128),
                    in_=w1_d[g])
            w2_sb = [w2p.tile([128, 16 * 2 * 128], F8, tag="w2",
                              name=f"w2_{m}")
                     for m in range(8)]
            for m in range(8):
                nc.scalar.dma_start(
                    out=w2_sb[m].rearrange("p (j i c) -> p j i c",
                                           i=2, c=128),
                    in_=w2_d[m])

            # ---------------- phase 3: sparse attention ----------------
            # Both heads of an item share one [128, 2, 512] score PSUM tile,
            # one exp, one mask op. Normalization deferred: o_un + den rows
            # evacuated per stream; reciprocal batched per half.
            onorm = [qkvr.tile([128, N], F8, name=f"onorm{hh}")
                     for hh in range(2)]
            obounce = dram.tile([NCORES, 128, TOK], F8)
            orecvb = dram.tile([NCORES, 128, TOK], F8)
            orecv = x2p.tile([64, 8, 2, TOK], F8)
            o_un = [qkvr.tile([64, 512], BF, name=f"oun{k}")
                    for k in range(8)]
            # den rows live at partitions {0,32,64,96} (safe write offsets);
            # memset 1.0 so the reciprocal over unused partitions is benign
            den4 = [qkvr.tile([128, 512], FP, name=f"den{hh}")
                    for hh in range(2)]
            recip4 = [qkvr.tile([128, 512], BF, name=f"recip{hh}")
                      for hh in range(2)]
            for hh in range(2):
                nc.vector.memset(den4[hh], 1.0)
            with tc.tile_pool(name="sps", bufs=2, space="PSUM") as sps, \
                 tc.tile_pool(name="ops", bufs=4, space="PSUM") as ops, \
                 tc.tile_pool(name="ptp", bufs=6) as ptp, \
                 tc.tile_pool(name="nrm", bufs=2) as nrm:
                for pair in range(2):
                    cs = (2 * pair, 2 * pair + 1)
                    o_ps = {(c, h): ops.tile([65, 512], FP, tag="ops",
                                             name=f"ops{c}_{h}")
                            for c in cs for h in range(2)}
                    for c in cs:
                        items = sched[c]
                        for idx in range(len(items)):
                            kt, c0, c1, mk = items[idx]
                            q0 = 512 * c
                            w = c1 - c0
                            s_ps = sps.tile([128, 2, 512], FP, tag="sps")
                            for h in range(2):
                                nc.tensor.matmul(
                                    s_ps[:, h, 0:w],
                                    kT[64 * h:64 * h + 64,
                                       128 * kt:128 * kt + 128],
                                    qT[64 * h:64 * h + 64, q0 + c0:q0 + c1],
                                    start=True, stop=True,
                                    skip_group_check=True)
                            p_sb = ptp.tile([128, 2, 512], BF, tag="pt")
                            if w == 512:
                                nc.scalar.activation(out=p_sb[:, :, :],
                                                     in_=s_ps[:, :, :],
                                                     func=AF.Exp, scale=0.125)
                            else:
                                for h in range(2):
                                    nc.scalar.activation(
                                        out=p_sb[:, h, 0:w],
                                        in_=s_ps[:, h, 0:w],
                                        func=AF.Exp, scale=0.125)
                            if mk is not None:
                                mo = MASK_OFF[mk]
                                for h in range(2):
                                    nc.gpsimd.tensor_mul(
                                        p_sb[:, h, 0:128], p_sb[:, h, 0:128],
                                        mask_sb[:, mo + 128 * h:
                                                mo + 128 * h + 128])
                            for h in range(2):
                                nc.tensor.matmul(
                                    o_ps[(c, h)][:, c0:c1],
                                    vaug[kt][:, 65 * h:65 * h + 65],
                                    p_sb[:, h, 0:w], start=(idx == 0),
                                    stop=(idx == len(items) - 1),
                                    skip_group_check=True)
                    for c in cs:
                        for h in range(2):
                            k = 2 * c + h
                            r = 32 * (k % 4)
                            nc.scalar.copy(out=o_un[k],
                                           in_=o_ps[(c, h)][0:64, :])
                            nc.scalar.copy(out=den4[pair][r:r + 1, :],
                                           in_=o_ps[(c, h)][64:65, :])
                    # reciprocal as soon as a half's denominators are done
                    # (DVE is otherwise idle here); the rest of the
                    # normalization is deferred past all items so the ACT /
                    # gpsimd queues never convoy the next chunk's exps/masks
                    with nc.allow_low_precision(reason="softmax denom "
                                                "recip as bf16"):
                        nc.vector.reciprocal(out=recip4[pair],
                                             in_=den4[pair])
                # deferred normalization tail + staging
                for k in range(8):
                    hh, kk = k // 4, k % 4
                    cc, h = k // 2, k % 2
                    # partition_broadcast reads partition 0 of the TILE
                    # (AP partition offset ignored): stage the row first
                    rtmp = nrm.tile([1, 512], BF, tag="rtmp", bufs=2)
                    nc.scalar.copy(
                        out=rtmp, in_=recip4[hh][32 * kk:32 * kk + 1, :])
                    rbc = nrm.tile([64, 512], BF, tag="rbc", bufs=3)
                    nc.gpsimd.partition_broadcast(rbc, rtmp)
                    nc.vector.tensor_mul(
                        onorm[hh][64 * h:64 * h + 64,
                                  (512 * cc) % N:(512 * cc) % N + 512],
                        o_un[k], rbc)
                    if k % 4 == 3:
                        hh = k // 4
                        nc.sync.dma_start(
                            out=obounce[4 * hh:4 * hh + 4].rearrange(
                                "j p t -> p j t"),
                            in_=onorm[hh].rearrange("p (j t) -> p j t", t=TOK))

            if dbg_d is not None:
                nc.sync.dma_start(out=dbg_d[0], in_=qT)
                nc.sync.dma_start(out=dbg_d[1], in_=kT)
                nc.sync.dma_start(out=dbg_d[2], in_=vT)
                dbgon = qkvr.tile([128, 2 * N], BF)
                nc.vector.tensor_copy(out=dbgon[:, 0:N], in_=onorm[0])
                nc.vector.tensor_copy(out=dbgon[:, N:2 * N], in_=onorm[1])
                nc.sync.dma_start(out=dbg_d[3][:, 0:N], in_=dbgon[:, 0:N])
                nc.sync.dma_start(out=dbg_d[4][:, 0:N],
                                  in_=dbgon[:, N:2 * N])
                nc.sync.dma_start(out=dbg_d[5][0:64, 0:512], in_=o_un[0])
                nc.sync.dma_start(out=dbg_d[5][0:64, 512:1024], in_=o_un[1])
                nc.sync.dma_start(out=dbg_d[5][0:64, 1024:1536], in_=o_un[6])
                nc.sync.dma_start(out=dbg_d[5][0:64, 1536:2048], in_=o_un[7])
                dbgrc = qkvr.tile([128, 1024], BF)
                nc.vector.tensor_copy(out=dbgrc[:, 0:512], in_=den4[0])
                nc.vector.tensor_copy(out=dbgrc[:, 512:1024], in_=den4[1])
                nc.sync.dma_start(out=dbg_d[6][:, 0:1024], in_=dbgrc)
                nc.sync.dma_start(out=dbg_d[6][:, 1024:1536], in_=recip4[0])
                nc.sync.dma_start(out=dbg_d[6][:, 1536:2048], in_=recip4[1])
            if single:
                nc.sync.dma_start(out=orecvb[:], in_=obounce[:])
            else:
                nc.gpsimd.collective_compute(
                    "AllToAll", ALU.bypass,
                    replica_groups=[list(range(NCORES))],
                    ins=[obounce.opt()], outs=[orecvb.opt()])
            # unstage with DoubleRow partition fold: orecv[p, k, i, t]
            nc.sync.dma_start(
                out=orecv,
                in_=orecvb.rearrange("k (i p) t -> p k i t", i=2))
            if dbg_d is not None:
                dbgor = qkvr.tile([128, S], BF)
                for i in range(2):
                    nc.vector.tensor_copy(
                        out=dbgor[64 * i:64 * i + 64, :].rearrange(
                            "p (k t) -> p k t", t=TOK),
                        in_=orecv[:, :, i, :])
                nc.sync.dma_start(out=dbg_d[7], in_=dbgor)

            # ---------------- phase 4: attn_out + residual -------------
            x2T = [x2p.tile([128, TOK], FP, name=f"x2T{m}") for m in range(8)]
            x2b = [x2p.tile([128, TOK], BF, name=f"x2b{m}") for m in range(8)]
            sqb = [x2p.tile([128, TOK], BF, name=f"sqb{m}") for m in range(8)]
            with tc.tile_pool(name="aops", bufs=3, space="PSUM") as aops, \
                 tc.tile_pool(name="aot", bufs=3) as aot:
                for g in range(2):
                    for mi in range(4):
                        m = 4 * g + mi
                        ps = aops.tile([128, TOK], FP, tag="aops")
                        for k in range(8):
                            nc.tensor.matmul(
                                ps, waor[g][:, mi, k],
                                orecv[:, k], perf_mode=DR,
                                start=(k == 0), stop=(k == 7))
                        ao_sb = aot.tile([128, TOK], FP, tag="ao")
                        nc.scalar.copy(out=ao_sb, in_=ps)
                        nc.vector.scalar_tensor_tensor(
                            out=x2T[m], in0=ao_sb,
                            scalar=gmsa_sb[:, m:m + 1],
                            in1=xsT[m], op0=ALU.mult, op1=ALU.add)
                        nc.vector.tensor_copy(out=x2b[m], in_=x2T[m])
                        nc.vector.tensor_mul(sqb[m], x2b[m], x2b[m])

            # ---------------- phase 5: LN2 (gamma/beta folded on host) -
            # h2 written as fp8 DoubleRow pairs: h2dr[j][:, i, :] = k-tile 2j+i
            h2dr = [x2p.tile([128, 2, TOK], F8, name=f"h2dr{j}")
                    for j in range(4)]
            with tc.tile_pool(name="l2ps", bufs=1, space="PSUM") as l2ps, \
                 tc.tile_pool(name="l2t", bufs=1) as l2t:
                sum_ps = l2ps.tile([1, TOK], FP, tag="l2sum")
                for k in range(8):
                    nc.tensor.matmul(sum_ps, ones_sb, x2b[k],
                                     start=(k == 0), stop=(k == 7))
                ssq_ps = l2ps.tile([1, TOK], FP, tag="l2ssq")
                for k in range(8):
                    nc.tensor.matmul(ssq_ps, ones_sb, sqb[k],
                                     start=(k == 0), stop=(k == 7),
                                     skip_group_check=True)
                mu2 = l2t.tile([1, TOK], BF)
                nc.vector.tensor_scalar_mul(mu2, sum_ps, 1.0 / D)
                mu2f = l2t.tile([1, TOK], FP)
                nc.vector.tensor_scalar_mul(mu2f, sum_ps, 1.0 / D)
                var2 = l2t.tile([1, TOK], FP)
                musq = l2t.tile([1, TOK], FP)
                nc.vector.tensor_mul(musq, mu2f, mu2f)
                nc.vector.tensor_scalar_mul(var2, ssq_ps, 1.0 / D)
                nc.vector.tensor_sub(var2, var2, musq)
                sd2 = l2t.tile([1, TOK], FP)
                nc.scalar.activation(out=sd2, in_=var2, func=AF.Sqrt,
                                     bias=eps1, scale=1.0)
                rstd2 = l2t.tile([1, TOK], BF)
                with nc.allow_low_precision(reason="rstd2 row as bf16 "
                                            "matmul-broadcast operand"):
                    nc.vector.reciprocal(out=rstd2, in_=sd2)
                # row broadcasts via PE rank-1 + ACT evac
                mu2bc_ps = l2ps.tile([128, TOK], FP, tag="l2mub")
                nc.tensor.matmul(mu2bc_ps, ones_row, mu2,
                                 start=True, stop=True,
                                 skip_group_check=True)
                rstd2bc_ps = l2ps.tile([128, TOK], FP, tag="l2rsb")
                nc.tensor.matmul(rstd2bc_ps, ones_row, rstd2,
                                 start=True, stop=True,
                                 skip_group_check=True)
                mu2bc = l2t.tile([128, TOK], FP)
                nc.scalar.copy(out=mu2bc, in_=mu2bc_ps)
                rstd2bc = l2t.tile([128, TOK], FP)
                nc.scalar.copy(out=rstd2bc, in_=rstd2bc_ps)
                for k in range(8):
                    u = l2t.tile([128, TOK], FP, tag="u", bufs=2)
                    nc.vector.tensor_sub(u, x2T[k], mu2bc)
                    nc.vector.tensor_mul(h2dr[k // 2][:, k % 2, :],
                                         u, rstd2bc)

            # ---------------- phase 6: MLP (fp8 DoubleRow) -------------
            g_dr = [gp.tile([128, 2, TOK], F8, name=f"g{j}")
                    for j in range(16)]
            w1r = [w1_sb[g].rearrange("p (mi j i c) -> p mi j i c",
                                      j=4, i=2, c=128) for g in range(8)]
            with tc.tile_pool(name="m1ps", bufs=3, space="PSUM") as m1ps:
                for g in range(8):
                    for mi in range(4):
                        m = 4 * g + mi
                        ps = m1ps.tile([128, TOK], FP, tag="m1")
                        for j in range(4):
                            nc.tensor.matmul(ps, w1r[g][:, mi, j],
                                             h2dr[j], perf_mode=DR,
                                             start=(j == 0), stop=(j == 3))
                        gfunc = (AF.Identity if os.environ.get("DBG_NO_GELU")
                                 else AF.Gelu_apprx_tanh)
                        # psum holds W1SCALE * z: gelu(z + b1) via ACT scale
                        nc.scalar.activation(out=g_dr[m // 2][:, m % 2, :],
                                             in_=ps, func=gfunc,
                                             bias=b1_sb[:, m:m + 1],
                                             scale=1.0 / W1SCALE)

            w2r = [w2_sb[m].rearrange("p (j i c) -> p j i c", i=2, c=128)
                   for m in range(8)]
            with tc.tile_pool(name="m2ps", bufs=3, space="PSUM") as m2ps, \
                 tc.tile_pool(name="outp", bufs=3) as outp:
                for m in range(8):
                    ps = m2ps.tile([128, TOK], FP, tag="m2")
                    for j in range(16):
                        nc.tensor.matmul(ps, w2r[m][:, j], g_dr[j],
                                         perf_mode=DR,
                                         start=(j == 0), stop=(j == 15))
                    # psum = W2SCALE*(m - b2); evac: gmlp/W2SCALE * ps + gb2
                    mo = outp.tile([128, TOK], FP, tag="mo")
                    nc.scalar.activation(out=mo, in_=ps, func=AF.Identity,
                                         bias=gb2_sb[:, m:m + 1],
                                         scale=gmlp_sb[:, m:m + 1])
                    outT = outp.tile([128, TOK], FP, tag="outT")
                    nc.vector.tensor_add(outT, mo, x2T[m])
                    nc.sync.dma_start(out=out_d[m], in_=outT)

            w2p.release()
            w1p.release()
            waop.release()

    nc.compile()
    return nc


# ---------------------------------------------------------------------------
# host side
# ---------------------------------------------------------------------------

_NC = None


def _get_nc():
    global _NC
    if _NC is None:
        _NC = build_program()
    return _NC


def _mask01_tiles():
    """[128,128] multiplicative 0/1 masks in S^T orientation (rows=k,
    cols=q), each doubled for the 2-head layout: [diag x2 | strict x2 |
    incl x2]."""
    a = np.arange(128) // BS
    diag = (a[:, None] == a[None, :])
    strict = (a[None, :] > a[:, None])
    incl = (a[None, :] >= a[:, None])
    m = np.concatenate([diag, diag, strict, strict, incl, incl],
                       axis=1).astype(np.float32)
    return np.ascontiguousarray(m.astype(bf16))


def _tile4(wT, km, mm):
    """[K, M] -> (m, p, k, c) with arr[m, p, k, c] = wT[128k+p, 128m+c]."""
    return wT.reshape(km, 128, mm, 128).transpose(2, 1, 0, 3)


def _group(w4, gs):
    """(m, p, k, c) -> (g, p, m_in_g, k, c) groups of gs m-tiles."""
    mm, p, km, c = w4.shape
    return np.ascontiguousarray(
        w4.reshape(mm // gs, gs, p, km, c).transpose(0, 2, 1, 3, 4)
        .astype(bf16))


def _prep_inputs(x, c, cos, sin, norm1_w, qkv_w, attn_out_w, norm2_w,
                 mlp_w1, mlp_b1, mlp_w2, mlp_b2, adaLN_w, adaLN_b):
    f32 = np.float32
    x = np.asarray(x, f32).reshape(S, D)
    c = np.asarray(c, f32).reshape(COND)
    cos = np.asarray(cos, f32)
    sin = np.asarray(sin, f32)
    qkv_w = np.asarray(qkv_w, f32)
    mlp_w1 = np.asarray(mlp_w1, f32)

    # adaLN modulation on host
    mods = adaLN_w.astype(f32) @ c + np.asarray(adaLN_b, f32)
    sh_msa, sc_msa, g_msa, sh_mlp, sc_mlp, g_mlp = mods.reshape(6, D)

    gam1 = (1.0 + sc_msa) * np.asarray(norm1_w, f32)          # [D]
    qkv_ws = qkv_w * gam1[None, :]                            # [3D, D]
    u_qkv = qkv_ws.sum(axis=1)                                # [3D]
    b_qkv = qkv_w @ sh_msa                                    # [3D]

    gam2 = (1.0 + sc_mlp) * np.asarray(norm2_w, f32)          # [D]
    w1s = mlp_w1 * gam2[None, :]                              # [4D, D]
    b1f = np.asarray(mlp_b1, f32) + mlp_w1 @ sh_mlp           # [4D]
    b2 = np.asarray(mlp_b2, f32)

    xb = x.astype(bf16)
    xT8 = np.ascontiguousarray(
        x.T.reshape(4, 2, 128, S).transpose(0, 2, 1, 3)
        .astype(fp8))                                         # (j, p, i, t)
    xTb = np.ascontiguousarray(xb.T.reshape(8, 128, S))       # (k, p, t)

    # rope tables expanded to S columns; sin table is DEST-signed for the
    # pure-swap pbs layout: rows 0:32 get -sin (they receive p[32:64]),
    # rows 32:64 get +sin. 1/WQSCALE folded in (fp8 weights are scaled up).
    cs = np.concatenate([cos, cos], axis=-1).T                # [64, N]
    ss = np.concatenate([-sin.T, sin.T], axis=0)              # [64, N]
    cos2 = np.tile(np.vstack([cs, cs]), (1, 2))               # [128, S]
    sin2 = np.tile(np.vstack([ss, ss]), (1, 2))               # [128, S]
    trig = np.ascontiguousarray(
        (np.hstack([cos2, sin2]) / WQSCALE).astype(bf16))

    waoT = np.ascontiguousarray(
        np.clip(np.asarray(attn_out_w, f32).T * WAOSCALE, -240, 240)
        .reshape(8, 2, 64, 2, 4, 128).transpose(3, 2, 4, 0, 1, 5)
        .astype(fp8))                         # (g, p, mi, k, i, c)
    # fp8 DoubleRow weights, scaled into e4m3 range (240 max on TRN)
    w1q = np.clip(_tile4(w1s.T, 8, 32) * W1SCALE, -240, 240)  # [32,128,8,128]
    w1T = np.ascontiguousarray(
        w1q.reshape(8, 4, 128, 4, 2, 128).transpose(0, 2, 1, 3, 4, 5)
        .astype(fp8))                               # (g, p, mi, j, i, c)
    w2q = np.clip(_tile4(np.asarray(mlp_w2, f32).T, 32, 8) * W2SCALE,
                  -240, 240)                        # [8, 128, 32, 128]
    w2T = np.ascontiguousarray(
        w2q.reshape(8, 128, 16, 2, 128).astype(fp8))  # (m, p, j, i, c)

    smallc = np.ascontiguousarray(np.hstack([
        (g_msa / WAOSCALE).reshape(8, 128).T,
        (g_mlp / W2SCALE).reshape(8, 128).T,
        b1f.reshape(32, 128).T,
        b2.reshape(8, 128).T,
        (g_mlp * b2).reshape(8, 128).T]).astype(f32))         # [128, 64]

    common = {
        "x": np.ascontiguousarray(xb),
        "xT": xT8,
        "waoT": waoT, "w1T": w1T, "w2T": w2T,
        "smallc": smallc, "trig": trig,
        "mask01": _mask01_tiles(),
    }
    in_maps = []
    for j in range(NCORES):
        wq = np.stack([
            np.clip(
                qkv_ws[s * D + 128 * j: s * D + 128 * j + 128].T
                * WQSCALE, -240, 240)
            .reshape(4, 2, 128, 128).transpose(0, 2, 1, 3)
            for s in range(3)])  # [3, j, p, i, c]
        wq = np.ascontiguousarray(
            wq.transpose(0, 2, 1, 3, 4).astype(fp8))  # (s, p, j, i, c)
        ub = np.stack([
            np.concatenate([u_qkv[s * D + 128 * j: s * D + 128 * j + 128]
                            for s in range(3)]),
            np.concatenate([b_qkv[s * D + 128 * j: s * D + 128 * j + 128]
                            for s in range(3)])]) * WQSCALE  # [2, 384]
        m = dict(common)
        m["wqkvT"] = wq  # [3, 128, 4, 2, 128] = (s, p, j, i, c)
        m["ubrow"] = np.ascontiguousarray(ub.astype(bf16))
        m["xsliceT"] = np.ascontiguousarray(
            xTb[:, :, TOK * j:TOK * j + TOK])
        in_maps.append(m)
    return in_maps


def _assemble(res):
    """Gather per-core feature-major outputs into the full [1, S, D]."""
    parts = []
    for j in range(NCORES):
        o = res.results[j]["out"]  # [8, 128, TOK] feature-major
        parts.append(np.ascontiguousarray(
            o.transpose(2, 0, 1).reshape(TOK, D)))
    return np.concatenate(parts, axis=0).reshape(1, S, D).astype(np.float32)


def kernel(**inputs):
    nc = _get_nc()
    in_maps = _prep_inputs(**inputs)
    res = run_bass_kernel_spmd(nc, in_maps, core_ids=list(range(NCORES)))
    return _assemble(res)



# revision 59
# speedup vs baseline: 1.1797x; 1.1797x over previous
"""DiT block with block-diffusion sparse attention on 8 Trainium2 NeuronCores.

v4 strategy (evolution of v3; see kernel_v3.py):
  - Clean-half-first software pipeline: per-512-token-quarter LN1 stats
    (bn_stats on fp8 token-major x) feed QKV chunks as they complete; the
    clean half (tokens 1024:2048) goes first so attention chunks c=2,3 can
    run interleaved with the noisy half's QKV matmuls on the PE.
  - rstd is NOT folded into the rope tables for k/v. Instead:
      k: per-partition scale at the softmax EXP (scores are [k, q] so
         rstd_k is a per-partition column; q's rstd*0.125 is folded into
         the q rope tables via one row-broadcast per quarter).
      v: per-partition scale on the V-transpose PSUM evacuation.
    This removes 2/3 of the rstd broadcast/fold work.
  - Stats avoid the DRAM bounce: per-tile [128,4] (negmu, sd, rstd/8) PE
    transposes into rows4[4, S]; rstd also kept token-tile-major in
    rstdc[128,16] for the exp/v scales.
  - rotate-half swap copies on GpSimd; attention mask multiplies on DVE
    (idle during attention); softmax reciprocal via reciprocal_approx_fast.
  - Softmax denominators/normalization per half: the clean half is
    normalized and staged to DRAM while the noisy half's attention still
    runs; only the noisy half's norm is exposed before the AllToAll.
  - Single consolidated DMAs with >=1KB descriptor lines; issues spread
    over the sync/tensor/gpsimd queues; w1/w2 prefetch gated behind the
    x loads via gpsimd program order.
  - attn_out uses full 128-partition stationary tiles (2 source cores per
    DoubleRow pair) - 2x fewer PE cycles than v3's 64-partition layout.
  - LN2 sum/sumsq matmuls interleaved with the attn_out chains; LN2 row
    broadcasts via stride-0 DMA instead of PE rank-1 + ACT evac.
"""

import os
import numpy as np
import ml_dtypes

import concourse.bass as bass
import concourse.tile as tile
from concourse import bacc, mybir
from concourse.bass_utils import run_bass_kernel_spmd
from concourse.masks import make_identity

bf16 = ml_dtypes.bfloat16
fp8 = ml_dtypes.float8_e4m3
FP = mybir.dt.float32
BF = mybir.dt.bfloat16
F8 = mybir.dt.float8e4
AF = mybir.ActivationFunctionType
ALU = mybir.AluOpType
DR = mybir.MatmulPerfMode.DoubleRow
WQSCALE = 64.0
WAOSCALE = 64.0
W1SCALE = 64.0
W2SCALE = 128.0

NCORES = 8
S, N, D, H, HD, BS, COND = 2048, 1024, 1024, 16, 64, 16, 128
TOK = S // NCORES  # 256 tokens per core after A2A

QORDER = [2, 3, 0, 1]  # clean half first


def _attn_schedule():
    """Per q-chunk list of (ktile, col0, col1, mask) in S^T orientation."""
    sched = []
    for c in range(4):
        items = []
        if c < 2:  # noisy q chunk
            for j in range(4 * c + 4):  # clean k tiles, bq > bk
                js = j - 4 * c
                if js < 0:
                    items.append((8 + j, 0, 512, None))
                else:
                    items.append((8 + j, 128 * js, 512, "strict"))
            for s in range(4):  # own-block diagonal (noisy k)
                items.append((4 * c + s, 128 * s, 128 * s + 128, "diag"))
        else:  # clean q chunk, bq >= bk
            cq = c - 2
            for j in range(4 * cq + 4):
                js = j - 4 * cq
                if js < 0:
                    items.append((8 + j, 0, 512, None))
                else:
                    items.append((8 + j, 128 * js, 512, "incl"))
        assert items[0][1] == 0 and items[0][2] == 512
        sched.append(items)
    return sched


MASK_OFF = {"diag": 0, "strict": 128, "incl": 256}


def build_program(single=False, dbg=False):
    nc = bacc.Bacc(
        "TRN2", target_bir_lowering=False, debug=False,
        enable_asserts=False, num_devices=1 if single else NCORES,
    )

    def din(name, shape, dt=BF):
        return nc.dram_tensor(name, shape, dt, kind="ExternalInput").ap()

    xst_d = din("xstat", [4, 128, 4, 1024])           # (q, p, sub, d)
    xT_d = din("xT", [4, 4, 128, 2, 512], F8)         # (n, j, p, i, t)
    trig_d = din("trig", [2, 128, 1024])              # cos128 | sin128(signed)
    mask01_d = din("mask01", [128, 384])              # diag|strict|incl
    wqkv_d = din("wqkvT", [128, 3, 4, 2, 128], F8)    # (p, s, j, i, c) scaled
    ub_d = din("ubrow", [2, 384])                     # (u; b) per-core slice
    xsT_d = din("xsliceT", [128, 8, 256])             # residual (p, k, t)
    wao_d = din("waoT", [128, 8, 4, 2, 128], F8)      # (p=(i,hd), m, j, i2, c)
    w1_d = din("w1T", [128, 8, 4, 4, 2, 128], F8)     # (p, g, mi, j, i, c)
    w2_d = din("w2T", [128, 8, 16, 2, 128], F8)       # (p, m, j, i, c)
    smallc_d = din("smallc", [128, 64], FP)           # gmsa|gmlp|b1'|b2|gb2
    out_d = nc.dram_tensor("out", [128, 8, TOK], FP, kind="ExternalOutput").ap()
    dbg_d = (nc.dram_tensor("dbg", [8, 128, S], BF,
                            kind="ExternalOutput").ap() if dbg else None)
    dbgf8_d = (nc.dram_tensor("dbgf8", [3, 128, S], F8,
                              kind="ExternalOutput").ap() if dbg else None)
    dbg32_d = (nc.dram_tensor("dbg32", [2, 128, S], FP,
                              kind="ExternalOutput").ap() if dbg else None)

    sched = _attn_schedule()

    with tile.TileContext(nc) as tc:
        with tc.tile_pool(name="const", bufs=1) as const, \
             tc.tile_pool(name="dram", bufs=1, space="DRAM") as dram, \
             tc.tile_pool(name="qkvr", bufs=1) as qkvr, \
             tc.tile_pool(name="vaugp", bufs=1) as vaugp, \
             tc.tile_pool(name="x2p", bufs=1) as x2p, \
             tc.tile_pool(name="gp", bufs=1) as gp:

            # ---------------- DMA issues ------------------------------
            # long-lived weight pools first (pool releases are LIFO)
            waop = tc.alloc_tile_pool(name="waop", bufs=1)
            w1p = tc.alloc_tile_pool(name="w1p", bufs=1)
            w2p = tc.alloc_tile_pool(name="w2p", bufs=1)
            xstp = tc.alloc_tile_pool(name="xstp", bufs=2)
            xTp = tc.alloc_tile_pool(name="xTp", bufs=1)
            xst_sb = {}
            xTn_sb = {}
            for q in QORDER:
                xst_sb[q] = xstp.tile([128, 4, 1024], BF, tag="xst",
                                      name=f"xst{q}")
                xTn_sb[q] = xTp.tile([128, 4, 2, 512], F8, name=f"xTn{q}")

            ub_sb = const.tile([2, 384], BF)
            trig_sb = const.tile([128, 2, 1024], BF)
            mask_sb = const.tile([128, 384], BF)
            smallc = const.tile([128, 64], FP)
            xsT = const.tile([128, 8, 256], BF)

            # sync queue: x stats quarters + small constants
            nc.sync.dma_start(out=xst_sb[2], in_=xst_d[2])
            nc.sync.dma_start(out=ub_sb, in_=ub_d)
            nc.sync.dma_start(out=xst_sb[3], in_=xst_d[3])
            nc.sync.dma_start(out=trig_sb,
                              in_=trig_d.rearrange("v p t -> p v t"))
            nc.sync.dma_start(out=xst_sb[0], in_=xst_d[0])
            nc.sync.dma_start(out=mask_sb, in_=mask01_d)
            nc.sync.dma_start(out=xst_sb[1], in_=xst_d[1])
            nc.sync.dma_start(out=smallc, in_=smallc_d)
            nc.sync.dma_start(out=xsT, in_=xsT_d)

            # scalar queue: xT chunks + qkv weights (before first sqrt)
            wq_sb = const.tile([128, 3, 4, 2, 128], F8)
            nc.scalar.dma_start(out=xTn_sb[2], in_=xT_d[2].rearrange("j p i t -> p j i t"))
            nc.scalar.dma_start(
                out=wq_sb, in_=wqkv_d)
            nc.scalar.dma_start(out=xTn_sb[3], in_=xT_d[3].rearrange("j p i t -> p j i t"))
            nc.scalar.dma_start(out=xTn_sb[0], in_=xT_d[0].rearrange("j p i t -> p j i t"))
            nc.scalar.dma_start(out=xTn_sb[1], in_=xT_d[1].rearrange("j p i t -> p j i t"))

            gmsa_sb = smallc[:, 0:8]
            gmlp_sb = smallc[:, 8:16]
            b1_sb = smallc[:, 16:48]
            b2_sb = smallc[:, 48:56]
            gb2_sb = smallc[:, 56:64]
            cosT = trig_sb[:, 0, :]
            sinT = trig_sb[:, 1, :]

            ones_sb = const.tile([128, 1], BF)
            nc.vector.memset(ones_sb, 1.0)
            eps128 = const.tile([128, 1], FP)
            nc.vector.memset(eps128, 1e-5)
            eps1 = const.tile([1, 1], FP)
            nc.vector.memset(eps1, 1e-5)
            ident_f = const.tile([128, 128], FP)
            make_identity(nc, ident_f)
            ident_b = const.tile([128, 128], BF)
            nc.vector.tensor_copy(out=ident_b, in_=ident_f)

            # stats / fold outputs (r8row separate: partition_broadcast
            # reads partition 0, rank-2 moving needs base partition 0)
            rows4 = const.tile([4, S], BF)        # negmu | sd | rstd/8 | pad
            r8row = const.tile([1, S], BF)        # rstd/8 at partition 0
            rtmp = const.tile([1, 512], BF)       # bcast staging row
            rstdc = const.tile([128, 16], FP)     # per token-tile rstd col

            qT = qkvr.tile([128, S], BF)
            kT = qkvr.tile([128, S], BF)
            vT = qkvr.tile([128, S], BF)
            qkv_dst = [qT, kT, vT]
            vaug = [vaugp.tile([128, 130], BF, name=f"vaug{kt}")
                    for kt in range(16)]

            onorm = [qkvr.tile([128, N], F8, name=f"onorm{hh}")
                     for hh in range(2)]
            obounce = dram.tile([NCORES, 128, TOK], F8)
            orecvb = dram.tile([NCORES, 128, TOK], F8)
            orecv2 = x2p.tile([128, 8, TOK], F8)
            ounp = tc.alloc_tile_pool(name="ounp", bufs=4)
            o_un = {}
            den4 = [qkvr.tile([128, 512], FP, name=f"den{hh}")
                    for hh in range(2)]
            recip4 = qkvr.tile([128, 512], FP, name="recip4")
            for hh in range(2):
                nc.vector.memset(den4[hh], 1.0)

            # attn_out weights prefetch (gated on gpsimd order below)
            wao_sb = waop.tile([128, 8 * 4 * 2 * 128], F8, name="wao")
            w1_sb = w1p.tile([128, 8 * 4 * 4 * 2 * 128], F8, name="w1")
            w2_sb = w2p.tile([128, 8 * 16 * 2 * 128], F8, name="w2")
            w2r = w2_sb.rearrange("p (m j i c) -> p m j i c", m=8, j=16,
                                  i=2, c=128)
            waor = wao_sb.rearrange("p (m j i c) -> p m j i c",
                                    m=8, j=4, i=2, c=128)
            w1r = w1_sb.rearrange("p (g mi j i c) -> p g mi j i c",
                                  g=8, mi=4, j=4, i=2, c=128)

            statp = tc.alloc_tile_pool(name="statp", bufs=4)
            qtab = tc.alloc_tile_pool(name="qtab", bufs=2)
            ropep = tc.alloc_tile_pool(name="ropep", bufs=2)
            rbp = tc.alloc_tile_pool(name="rbp", bufs=1)
            qtabs = {}
            mmps = tc.alloc_tile_pool(name="mmps", bufs=2, space="PSUM")
            sps = tc.alloc_tile_pool(name="sps", bufs=2, space="PSUM")
            ops = tc.alloc_tile_pool(name="ops", bufs=2, space="PSUM")

            wqr = wq_sb

            # ---------------- phase builders --------------------------
            def stats(q):
                for sub in range(4):
                    ti = 4 * q + sub
                    x_sb = xst_sb[q][:, sub, :]
                    st = statp.tile([128, 2, 6], FP, tag="bst")
                    for sg in range(2):
                        nc.vector.bn_stats(
                            out=st[:, sg, :],
                            in_=x_sb[:, 512 * sg:512 * sg + 512])
                    mv = statp.tile([128, 2], FP, tag="mv")
                    nc.vector.bn_aggr(out=mv, in_=st)
                    # col 32 -> transposed row at partition 32 (ACT reads
                    # must start at a multiple-of-32 partition)
                    st4 = statp.tile([128, 33], FP, tag="st4")
                    nc.vector.tensor_scalar_mul(
                        st4[:, 0:1], mv[:, 0:1], -1.0)
                    nc.scalar.activation(
                        out=st4[:, 1:2], in_=mv[:, 1:2],
                        func=AF.Sqrt, bias=eps128, scale=1.0)
                    nc.vector.reciprocal(
                        out=rstdc[:, ti:ti + 1], in_=st4[:, 1:2])
                    nc.vector.tensor_scalar_mul(
                        st4[:, 32:33], rstdc[:, ti:ti + 1], 0.125)
                    ps = mmps.tile([128, 512], FP, tag="mm", name="stps")
                    nc.tensor.transpose(ps[0:33, 0:128], st4, ident_f)
                    nc.scalar.copy(out=rows4[0:2, 128 * ti:128 * ti + 128],
                                   in_=ps[0:2, 0:128])
                    nc.scalar.copy(out=r8row[:, 128 * ti:128 * ti + 128],
                                   in_=ps[32:33, 0:128])

            def fold(q):
                nsl = slice(512 * q, 512 * q + 512)
                nmod = slice(512 * (q % 2), 512 * (q % 2) + 512)
                rb = rbp.tile([128, 512], BF, tag="rb")
                nc.gpsimd.partition_broadcast(rb, r8row[:, nsl])
                cq = qtab.tile([128, 512], BF, tag="cosq", name=f"cosq{q}")
                sq = qtab.tile([128, 512], BF, tag="sinq", name=f"sinq{q}")
                nc.vector.tensor_mul(cq, cosT[:, nmod], rb)
                nc.vector.tensor_mul(sq, sinT[:, nmod], rb)
                qtabs[q] = (cq, sq)

            def qkv(n):
                nsl = slice(512 * n, 512 * n + 512)
                nmod = slice(512 * (n % 2), 512 * (n % 2) + 512)
                for m in (1, 2, 0):  # q last: more slack for the fold
                    ps = mmps.tile([128, 512], FP, tag="mm", name="qkvps")
                    for j in range(4):
                        nc.tensor.matmul(
                            ps, wqr[:, m, j],
                            xTn_sb[n][:, j], perf_mode=DR,
                            start=(j == 0), stop=False,
                            skip_group_check=True)
                    nc.tensor.matmul(
                        ps, ub_sb[:, 128 * m:128 * m + 128],
                        rows4[0:2, nsl], start=False, stop=True,
                        skip_group_check=True)
                    pb = ropep.tile([128, 512], BF, tag="pb")
                    nc.scalar.copy(out=pb, in_=ps)
                    pbs = ropep.tile([128, 512], BF, tag="pbs")
                    for h in range(2):
                        r = 64 * h
                        nc.vector.tensor_copy(
                            out=pbs[r:r + 32, :], in_=pb[r + 32:r + 64, :])
                        nc.vector.tensor_copy(
                            out=pbs[r + 32:r + 64, :], in_=pb[r:r + 32, :])
                    ca = qtabs[n][0] if m == 0 else cosT[:, nmod]
                    sa = qtabs[n][1] if m == 0 else sinT[:, nmod]
                    t1 = ropep.tile([128, 512], BF, tag="t1")
                    nc.vector.tensor_mul(t1, pb, ca)
                    nc.vector.tensor_mul(pbs, pbs, sa)
                    nc.vector.tensor_add(qkv_dst[m][:, nsl], t1, pbs)

            def vaug_blk(n):
                for kt in range(4 * n, 4 * n + 4):
                    ps = mmps.tile([128, 512], FP, tag="mm", name="vtps")
                    psb = ps.bitcast(BF)
                    nc.tensor.transpose(
                        psb[:, 0:128], vT[:, 128 * kt:128 * kt + 128],
                        ident_b)
                    va = vaug[kt]
                    nc.vector.memset(va[:, 64:65], 1.0)
                    nc.vector.memset(va[:, 129:130], 1.0)
                    # v gets its token's rstd here (per-partition scale)
                    nc.scalar.activation(
                        out=va[:, 0:130].rearrange(
                            "p (h y) -> p h y", y=65)[:, :, 0:64],
                        in_=psb[:, 0:128].rearrange("p (h d) -> p h d", d=64),
                        func=AF.Copy, scale=rstdc[:, kt:kt + 1])

            def attn(c):
                items = sched[c]
                nit = len(items)
                o_ps = {h: ops.tile([65, 512], FP, tag="ops",
                                    name=f"ops{c}_{h}")
                        for h in range(2)}
                q0 = 512 * c
                s_tiles = {}

                def score(idx):
                    kt, c0, c1, mk = items[idx]
                    w = c1 - c0
                    s_ps = sps.tile([128, 2, 512], FP, tag="sps")
                    s_tiles[idx] = s_ps
                    for h in range(2):
                        nc.tensor.matmul(
                            s_ps[:, h, 0:w],
                            kT[64 * h:64 * h + 64,
                               128 * kt:128 * kt + 128],
                            qT[64 * h:64 * h + 64, q0 + c0:q0 + c1],
                            start=True, stop=True,
                            skip_group_check=True)

                def finish(idx):
                    kt, c0, c1, mk = items[idx]
                    w = c1 - c0
                    s_ps = s_tiles.pop(idx)
                    p_sb = ropep.tile([128, 2, 512], BF, tag="pt")
                    sc = rstdc[:, kt:kt + 1]
                    if w == 512:
                        nc.scalar.activation(out=p_sb[:, :, :],
                                             in_=s_ps[:, :, :],
                                             func=AF.Exp, scale=sc)
                    else:
                        for h in range(2):
                            nc.scalar.activation(
                                out=p_sb[:, h, 0:w],
                                in_=s_ps[:, h, 0:w],
                                func=AF.Exp, scale=sc)
                    if mk is not None:
                        mo = MASK_OFF[mk]
                        for h in range(2):
                            nc.vector.tensor_mul(
                                p_sb[:, h, 0:128], p_sb[:, h, 0:128],
                                mask_sb[:, mo:mo + 128])
                    for h in range(2):
                        nc.tensor.matmul(
                            o_ps[h][:, c0:c1],
                            vaug[kt][:, 65 * h:65 * h + 65],
                            p_sb[:, h, 0:w], start=(idx == 0),
                            stop=(idx == nit - 1),
                            skip_group_check=True)

                score(0)
                for idx in range(1, nit):
                    score(idx)
                    finish(idx - 1)
                finish(nit - 1)
                hh = 1 if c >= 2 else 0
                for h in range(2):
                    k = 2 * c + h
                    r = 32 * (k % 4)
                    o_un[k] = ounp.tile([64, 512], BF, tag="oun",
                                        name=f"oun{k}")
                    nc.scalar.copy(out=o_un[k], in_=o_ps[h][0:64, :])
                    nc.scalar.copy(out=den4[hh][r:r + 1, :],
                                   in_=o_ps[h][64:65, :])

            def norm_half(hh):
                # streams of this half: chunks (2,3) for hh=1, (0,1) for 0
                nc.vector.reciprocal_approx_fast(
                    out=recip4, in_=den4[hh])
                for c in ((2, 3) if hh else (0, 1)):
                    for h in range(2):
                        k = 2 * c + h
                        r = 32 * (k % 4)
                        rbc = ropep.tile([128, 512], BF, tag="t1",
                                         name="rbc")
                        nc.scalar.copy(out=rtmp, in_=recip4[r:r + 1, :])
                        nc.gpsimd.partition_broadcast(rbc[0:64, :], rtmp)
                        nc.vector.tensor_mul(
                            onorm[hh][64 * h:64 * h + 64,
                                      (512 * c) % N:(512 * c) % N + 512],
                            o_un[k], rbc[0:64, :])
                nc.sync.dma_start(
                    out=obounce[4 * hh:4 * hh + 4].rearrange(
                        "j p t -> p j t"),
                    in_=onorm[hh].rearrange("p (j t) -> p j t", t=TOK))

            # ---------------- emission order --------------------------
            stats(2)
            fold(2)
            stats(3)
            fold(3)
            qkv(2)
            vaug_blk(2)
            qkv(3)
            vaug_blk(3)
            stats(0)
            fold(0)
            stats(1)
            fold(1)
            attn(2)
            # gpsimd order gate: issue weight prefetches here (after the
            # fold broadcasts / early swaps, before the late swaps)
            nc.gpsimd.dma_start(out=wao_sb.rearrange(
                "p (m j i c) -> p m j i c", m=8, j=4, i=2, c=128),
                in_=wao_d)
            nc.gpsimd.dma_start(out=w1_sb.rearrange(
                "p (g mi j i c) -> p g mi j i c", g=8, mi=4, j=4, i=2,
                c=128), in_=w1_d)
            qkv(0)
            vaug_blk(0)
            attn(3)
            norm_half(1)
            qkv(1)
            vaug_blk(1)
            attn(0)
            attn(1)
            norm_half(0)

            if dbg_d is not None:
                nc.sync.dma_start(out=dbg_d[0], in_=qT)
                nc.sync.dma_start(out=dbg_d[1], in_=kT)
                nc.sync.dma_start(out=dbg_d[2], in_=vT)
                for hh in range(2):
                    nc.sync.dma_start(out=dbgf8_d[0][:, N * hh:N * hh + N],
                                      in_=onorm[hh])
                    nc.sync.dma_start(
                        out=dbg32_d[0][:, 512 * hh:512 * hh + 512],
                        in_=den4[hh])
                nc.sync.dma_start(out=dbg32_d[0][:, 1024:1040], in_=rstdc)
                nc.sync.dma_start(out=dbg_d[5][0:4, :], in_=rows4)
                nc.sync.dma_start(out=dbg_d[6][0:1, :], in_=r8row)

            # w2 prefetch issue late on gpsimd (x loads long done)
            nc.gpsimd.dma_start(out=w2_sb.rearrange(
                "p (m j i c) -> p m j i c", m=8, j=16, i=2, c=128),
                in_=w2_d)

            if single:
                nc.sync.dma_start(out=orecvb[:], in_=obounce[:])
            else:
                nc.gpsimd.collective_compute(
                    "AllToAll", ALU.bypass,
                    replica_groups=[list(range(NCORES))],
                    ins=[obounce.opt()], outs=[orecvb.opt()])
            nc.sync.dma_start(
                out=orecv2, in_=orecvb.rearrange("k r t -> r k t"))

            ops.release()
            sps.release()
            mmps.release()
            rbp.release()
            ropep.release()
            qtab.release()
            statp.release()
            ounp.release()
            xTp.release()
            xstp.release()

            # ---------------- phase 4: attn_out + residual + LN2 stats -
            x2T = [x2p.tile([128, TOK], FP, name=f"x2T{m}") for m in range(8)]
            x2b = [x2p.tile([128, TOK], BF, name=f"x2b{m}") for m in range(8)]
            sqb = [x2p.tile([128, TOK], BF, name=f"sqb{m}") for m in range(8)]
            h2dr = [x2p.tile([128, 2, TOK], F8, name=f"h2dr{j}")
                    for j in range(4)]
            with tc.tile_pool(name="aops", bufs=3, space="PSUM") as aops, \
                 tc.tile_pool(name="l2ps", bufs=1, space="PSUM") as l2ps, \
                 tc.tile_pool(name="aot", bufs=3) as aot, \
                 tc.tile_pool(name="l2t", bufs=1) as l2t:
                sum_ps = l2ps.tile([1, TOK], FP, tag="l2sum")
                ssq_ps = l2ps.tile([1, TOK], FP, tag="l2ssq")

                def ao_chain(m):
                    ps = aops.tile([128, TOK], FP, tag="aops")
                    for j in range(4):
                        nc.tensor.matmul(
                            ps, waor[:, m, j],
                            orecv2[:, 2 * j:2 * j + 2, :], perf_mode=DR,
                            start=(j == 0), stop=(j == 3),
                            skip_group_check=True)
                    ao_sb = aot.tile([128, TOK], FP, tag="ao")
                    nc.scalar.copy(out=ao_sb, in_=ps)
                    nc.vector.scalar_tensor_tensor(
                        out=x2T[m], in0=ao_sb,
                        scalar=gmsa_sb[:, m:m + 1],
                        in1=xsT[:, m, :], op0=ALU.mult, op1=ALU.add)
                    nc.vector.tensor_copy(out=x2b[m], in_=x2T[m])
                    nc.vector.tensor_mul(sqb[m], x2b[m], x2b[m])

                def ln2_acc(m):
                    nc.tensor.matmul(sum_ps, ones_sb, x2b[m],
                                     start=(m == 0), stop=(m == 7),
                                     skip_group_check=True)
                    nc.tensor.matmul(ssq_ps, ones_sb, sqb[m],
                                     start=(m == 0), stop=(m == 7),
                                     skip_group_check=True)

                ao_chain(0)
                for m in range(1, 8):
                    ao_chain(m)
                    ln2_acc(m - 1)
                ln2_acc(7)

                mu2f = l2t.tile([1, TOK], FP)
                nc.vector.tensor_scalar_mul(mu2f, sum_ps, 1.0 / D)
                var2 = l2t.tile([1, TOK], FP)
                musq = l2t.tile([1, TOK], FP)
                nc.vector.tensor_mul(musq, mu2f, mu2f)
                nc.vector.tensor_scalar_mul(var2, ssq_ps, 1.0 / D)
                nc.vector.tensor_sub(var2, var2, musq)
                sd2 = l2t.tile([1, TOK], FP)
                nc.scalar.activation(out=sd2, in_=var2, func=AF.Sqrt,
                                     bias=eps1, scale=1.0)
                rstd2 = l2t.tile([1, TOK], FP)
                nc.vector.reciprocal_approx_fast(out=rstd2, in_=sd2)
                # row broadcasts (sources live at partition 0)
                mu2bc = l2t.tile([128, TOK], FP)
                nc.gpsimd.partition_broadcast(mu2bc, mu2f)
                rstd2bc = l2t.tile([128, TOK], FP)
                nc.gpsimd.partition_broadcast(rstd2bc, rstd2)
                for k in range(8):
                    u = l2t.tile([128, TOK], FP, tag="u", bufs=2)
                    nc.vector.tensor_sub(u, x2T[k], mu2bc)
                    nc.vector.tensor_mul(h2dr[k // 2][:, k % 2, :],
                                         u, rstd2bc)

            if dbg_d is not None:
                nc.sync.dma_start(
                    out=dbgf8_d[1].rearrange("p (k t) -> p k t", t=TOK),
                    in_=orecv2)
                for m in range(8):
                    nc.sync.dma_start(
                        out=dbg32_d[1][:, TOK * m:TOK * m + TOK],
                        in_=x2T[m])
                for j in range(4):
                    nc.sync.dma_start(
                        out=dbgf8_d[2].rearrange(
                            "p (j i t) -> p j i t", j=4, i=2)[:, j],
                        in_=h2dr[j])

            # ---------------- phase 6: MLP (fp8 DoubleRow) -------------
            g_dr = gp.tile([128, 16, 2, TOK], F8, name="g_dr")
            with tc.tile_pool(name="m1ps", bufs=3, space="PSUM") as m1ps:
                for g in range(8):
                    for mi in range(4):
                        m = 4 * g + mi
                        ps = m1ps.tile([128, TOK], FP, tag="m1")
                        for j in range(4):
                            nc.tensor.matmul(ps, w1r[:, g, mi, j],
                                             h2dr[j], perf_mode=DR,
                                             start=(j == 0), stop=(j == 3))
                        gfunc = (AF.Identity if os.environ.get("DBG_NO_GELU")
                                 else AF.Gelu_apprx_tanh)
                        nc.scalar.activation(out=g_dr[:, m // 2, m % 2, :],
                                             in_=ps, func=gfunc,
                                             bias=b1_sb[:, m:m + 1],
                                             scale=1.0 / W1SCALE)

            outT = gp.tile([128, 8, TOK], FP, name="outT")
            with tc.tile_pool(name="m2ps", bufs=3, space="PSUM") as m2ps, \
                 tc.tile_pool(name="outp", bufs=3) as outp:
                for m in range(8):
                    ps = m2ps.tile([128, TOK], FP, tag="m2")
                    for j in range(16):
                        nc.tensor.matmul(ps, w2r[:, m, j], g_dr[:, j],
                                         perf_mode=DR,
                                         start=(j == 0), stop=(j == 15))
                    # psum = W2SCALE*(m - b2); evac: gmlp/W2SCALE * ps + gb2
                    mo = outp.tile([128, TOK], FP, tag="mo")
                    nc.scalar.activation(out=mo, in_=ps, func=AF.Identity,
                                         bias=gb2_sb[:, m:m + 1],
                                         scale=gmlp_sb[:, m:m + 1])
                    nc.vector.tensor_add(outT[:, m, :], mo, x2T[m])
                nc.sync.dma_start(out=out_d, in_=outT)
            w2p.release()
            w1p.release()
            waop.release()

    nc.compile()
    return nc


# ---------------------------------------------------------------------------
# host side
# ---------------------------------------------------------------------------

_NC = None


def _get_nc():
    global _NC
    if _NC is None:
        _NC = build_program()
    return _NC


def _mask01_tiles():
    a = np.arange(128) // BS
    diag = (a[:, None] == a[None, :])
    strict = (a[None, :] > a[:, None])
    incl = (a[None, :] >= a[:, None])
    m = np.concatenate([diag, strict, incl],
                       axis=1).astype(np.float32)
    return np.ascontiguousarray(m.astype(bf16))


def _tile4(wT, km, mm):
    """[K, M] -> (m, p, k, c) with arr[m, p, k, c] = wT[128k+p, 128m+c]."""
    return wT.reshape(km, 128, mm, 128).transpose(2, 1, 0, 3)


def _prep_inputs(x, c, cos, sin, norm1_w, qkv_w, attn_out_w, norm2_w,
                 mlp_w1, mlp_b1, mlp_w2, mlp_b2, adaLN_w, adaLN_b):
    f32 = np.float32
    x = np.asarray(x, f32).reshape(S, D)
    c = np.asarray(c, f32).reshape(COND)
    cos = np.asarray(cos, f32)
    sin = np.asarray(sin, f32)
    qkv_w = np.asarray(qkv_w, f32)
    mlp_w1 = np.asarray(mlp_w1, f32)

    # adaLN modulation on host
    mods = adaLN_w.astype(f32) @ c + np.asarray(adaLN_b, f32)
    sh_msa, sc_msa, g_msa, sh_mlp, sc_mlp, g_mlp = mods.reshape(6, D)

    gam1 = (1.0 + sc_msa) * np.asarray(norm1_w, f32)          # [D]
    qkv_ws = qkv_w * gam1[None, :]                            # [3D, D]
    u_qkv = qkv_ws.sum(axis=1)                                # [3D]
    b_qkv = qkv_w @ sh_msa                                    # [3D]

    gam2 = (1.0 + sc_mlp) * np.asarray(norm2_w, f32)          # [D]
    w1s = mlp_w1 * gam2[None, :]                              # [4D, D]
    b1f = np.asarray(mlp_b1, f32) + mlp_w1 @ sh_mlp           # [4D]
    b2 = np.asarray(mlp_b2, f32)

    xb = x.astype(bf16)
    # stats copy: (q, p, sub, d) fp8
    xst = np.ascontiguousarray(
        x.reshape(4, 4, 128, D).transpose(0, 2, 1, 3).astype(bf16))
    # qkv moving: (n, j, p, i, t) fp8
    xT8 = np.ascontiguousarray(
        x.T.reshape(4, 2, 128, 4, 512).transpose(3, 0, 2, 1, 4)
        .astype(fp8))
    xTb = xb.T.reshape(8, 128, S)                             # (k, p, t)

    # rope tables [2, 128, 1024]: cos | dest-signed sin; 1/WQSCALE folded.
    cs = np.concatenate([cos, cos], axis=-1).T                # [64, N]
    ss = np.concatenate([-sin.T, sin.T], axis=0)              # [64, N]
    cos128 = np.vstack([cs, cs])                              # [128, N]
    sin128 = np.vstack([ss, ss])
    trig = np.ascontiguousarray(
        (np.stack([cos128, sin128]) / WQSCALE).astype(bf16))

    # attn_out: (p=(i,hd), m, j, i2, c); f = (2*(2j+i2)+i)*64 + hd
    waoT = np.ascontiguousarray(
        np.clip(np.asarray(attn_out_w, f32).T * WAOSCALE, -240, 240)
        .reshape(4, 2, 2, 64, 8, 128).transpose(2, 3, 4, 0, 1, 5)
        .reshape(128, 8, 4, 2, 128).astype(fp8))
    # w1: (p, g, mi, j, i, c)
    w1q = np.clip(_tile4(w1s.T, 8, 32) * W1SCALE, -240, 240)  # [32,128,8,128]
    w1T = np.ascontiguousarray(
        w1q.reshape(8, 4, 128, 4, 2, 128).transpose(2, 0, 1, 3, 4, 5)
        .astype(fp8))                                 # (p, g, mi, j, i, c)
    # w2: (p, m, j, i, c)
    w2q = np.clip(_tile4(np.asarray(mlp_w2, f32).T, 32, 8) * W2SCALE,
                  -240, 240)                          # [8, 128, 32, 128]
    w2T = np.ascontiguousarray(
        w2q.reshape(8, 128, 16, 2, 128).transpose(1, 0, 2, 3, 4)
        .astype(fp8))

    smallc = np.ascontiguousarray(np.hstack([
        (g_msa / WAOSCALE).reshape(8, 128).T,
        (g_mlp / W2SCALE).reshape(8, 128).T,
        b1f.reshape(32, 128).T,
        b2.reshape(8, 128).T,
        (g_mlp * b2).reshape(8, 128).T]).astype(f32))         # [128, 64]

    common = {
        "xstat": xst,
        "xT": xT8,
        "waoT": waoT, "w1T": w1T, "w2T": w2T,
        "smallc": smallc, "trig": trig,
        "mask01": _mask01_tiles(),
    }
    in_maps = []
    for j in range(NCORES):
        wq = np.stack([
            np.clip(
                qkv_ws[s * D + 128 * j: s * D + 128 * j + 128].T
                * WQSCALE, -240, 240)
            .reshape(4, 2, 128, 128).transpose(0, 2, 1, 3)
            for s in range(3)])  # [s, j, p, i, c]
        wq = np.ascontiguousarray(
            wq.transpose(2, 0, 1, 3, 4).astype(fp8))  # (p, s, j, i, c)
        ub = np.stack([
            np.concatenate([u_qkv[s * D + 128 * j: s * D + 128 * j + 128]
                            for s in range(3)]),
            np.concatenate([b_qkv[s * D + 128 * j: s * D + 128 * j + 128]
                            for s in range(3)])]) * WQSCALE  # [2, 384]
        m = dict(common)
        m["wqkvT"] = wq
        m["ubrow"] = np.ascontiguousarray(ub.astype(bf16))
        m["xsliceT"] = np.ascontiguousarray(
            xTb[:, :, TOK * j:TOK * j + TOK].transpose(1, 0, 2))  # (p,k,t)
        in_maps.append(m)
    return in_maps


def _assemble(res):
    """Gather per-core outputs [128, 8, TOK] (p, m, t) into [1, S, D]."""
    parts = []
    for j in range(NCORES):
        o = res.results[j]["out"]  # [128, 8, TOK]
        parts.append(np.ascontiguousarray(
            o.transpose(2, 1, 0).reshape(TOK, D)))
    return np.concatenate(parts, axis=0).reshape(1, S, D).astype(np.float32)


def kernel(**inputs):
    nc = _get_nc()
    in_maps = _prep_inputs(**inputs)
    res = run_bass_kernel_spmd(nc, in_maps, core_ids=list(range(NCORES)))
    return _assemble(res)


# revision 61
# speedup vs baseline: 1.3356x; 1.1322x over previous
"""DiT block with block-diffusion sparse attention on 8 Trainium2 NeuronCores.

v4 strategy (evolution of v3; see kernel_v3.py):
  - Clean-half-first software pipeline: per-512-token-quarter LN1 stats
    (bn_stats on fp8 token-major x) feed QKV chunks as they complete; the
    clean half (tokens 1024:2048) goes first so attention chunks c=2,3 can
    run interleaved with the noisy half's QKV matmuls on the PE.
  - rstd is NOT folded into the rope tables for k/v. Instead:
      k: per-partition scale at the softmax EXP (scores are [k, q] so
         rstd_k is a per-partition column; q's rstd*0.125 is folded into
         the q rope tables via one row-broadcast per quarter).
      v: per-partition scale on the V-transpose PSUM evacuation.
    This removes 2/3 of the rstd broadcast/fold work.
  - Stats avoid the DRAM bounce: per-tile [128,4] (negmu, sd, rstd/8) PE
    transposes into rows4[4, S]; rstd also kept token-tile-major in
    rstdc[128,16] for the exp/v scales.
  - rotate-half swap copies on GpSimd; attention mask multiplies on DVE
    (idle during attention); softmax reciprocal via reciprocal_approx_fast.
  - Softmax denominators/normalization per half: the clean half is
    normalized and staged to DRAM while the noisy half's attention still
    runs; only the noisy half's norm is exposed before the AllToAll.
  - Single consolidated DMAs with >=1KB descriptor lines; issues spread
    over the sync/tensor/gpsimd queues; w1/w2 prefetch gated behind the
    x loads via gpsimd program order.
  - attn_out uses full 128-partition stationary tiles (2 source cores per
    DoubleRow pair) - 2x fewer PE cycles than v3's 64-partition layout.
  - LN2 sum/sumsq matmuls interleaved with the attn_out chains; LN2 row
    broadcasts via stride-0 DMA instead of PE rank-1 + ACT evac.
"""

import os
import numpy as np
import ml_dtypes

import concourse.bass as bass
import concourse.tile as tile
from concourse import bacc, mybir
from concourse.bass_utils import run_bass_kernel_spmd
from concourse.masks import make_identity

bf16 = ml_dtypes.bfloat16
fp8 = ml_dtypes.float8_e4m3
FP = mybir.dt.float32
BF = mybir.dt.bfloat16
F8 = mybir.dt.float8e4
AF = mybir.ActivationFunctionType
ALU = mybir.AluOpType
DR = mybir.MatmulPerfMode.DoubleRow
WQSCALE = 64.0
WAOSCALE = 64.0
W1SCALE = 64.0
W2SCALE = 128.0

NCORES = 8
S, N, D, H, HD, BS, COND = 2048, 1024, 1024, 16, 64, 16, 128
TOK = S // NCORES  # 256 tokens per core after A2A

QORDER = [2, 3, 0, 1]  # clean half first


def _attn_schedule():
    """Per q-chunk list of (ktile, col0, col1, mask) in S^T orientation."""
    sched = []
    for c in range(4):
        items = []
        if c < 2:  # noisy q chunk
            for j in range(4 * c + 4):  # clean k tiles, bq > bk
                js = j - 4 * c
                if js < 0:
                    items.append((8 + j, 0, 512, None))
                else:
                    items.append((8 + j, 128 * js, 512, "strict"))
            for s in range(4):  # own-block diagonal (noisy k)
                items.append((4 * c + s, 128 * s, 128 * s + 128, "diag"))
        else:  # clean q chunk, bq >= bk
            cq = c - 2
            for j in range(4 * cq + 4):
                js = j - 4 * cq
                if js < 0:
                    items.append((8 + j, 0, 512, None))
                else:
                    items.append((8 + j, 128 * js, 512, "incl"))
        assert items[0][1] == 0 and items[0][2] == 512
        sched.append(items)
    return sched


MASK_OFF = {"diag": 0, "strict": 128, "incl": 256}


def build_program(single=False, dbg=False):
    nc = bacc.Bacc(
        "TRN2", target_bir_lowering=False, debug=False,
        enable_asserts=False, num_devices=1 if single else NCORES,
    )

    def din(name, shape, dt=BF):
        return nc.dram_tensor(name, shape, dt, kind="ExternalInput").ap()

    rows2_d = din("rows2", [2, S])                    # negmu | sd (host)
    r8_d = din("r8row", [1, S])                       # rstd/8 (host)
    rstdc_d = din("rstdc", [128, 16], FP)             # rstd token-tile cols
    xT_d = din("xT", [4, 4, 128, 2, 512], F8)         # (n, j, p, i, t)
    trig_d = din("trig", [2, 128, 1024])              # cos128 | sin128(signed)
    mask01_d = din("mask01", [128, 384])              # diag|strict|incl
    wqkv_d = din("wqkvT", [128, 3, 4, 2, 128], F8)    # (p, s, j, i, c) scaled
    ub_d = din("ubrow", [2, 384])                     # (u; b) per-core slice
    xsT_d = din("xsliceT", [128, 8, 256])             # residual (p, k, t)
    wao_d = din("waoT", [128, 8, 4, 2, 128], F8)      # (p=(i,hd), m, j, i2, c)
    w1_d = din("w1T", [128, 8, 4, 4, 2, 128], F8)     # (p, g, mi, j, i, c)
    w2_d = din("w2T", [128, 8, 16, 2, 128], F8)       # (p, m, j, i, c)
    smallc_d = din("smallc", [128, 64], FP)           # gmsa|gmlp|b1'|b2|gb2
    out_d = nc.dram_tensor("out", [128, 8, TOK], FP, kind="ExternalOutput").ap()
    dbg_d = (nc.dram_tensor("dbg", [8, 128, S], BF,
                            kind="ExternalOutput").ap() if dbg else None)
    dbgf8_d = (nc.dram_tensor("dbgf8", [3, 128, S], F8,
                              kind="ExternalOutput").ap() if dbg else None)
    dbg32_d = (nc.dram_tensor("dbg32", [2, 128, S], FP,
                              kind="ExternalOutput").ap() if dbg else None)

    sched = _attn_schedule()

    with tile.TileContext(nc) as tc:
        with tc.tile_pool(name="const", bufs=1) as const, \
             tc.tile_pool(name="dram", bufs=1, space="DRAM") as dram, \
             tc.tile_pool(name="qkvr", bufs=1) as qkvr, \
             tc.tile_pool(name="vaugp", bufs=1) as vaugp, \
             tc.tile_pool(name="x2p", bufs=1) as x2p, \
             tc.tile_pool(name="gp", bufs=1) as gp:

            # ---------------- DMA issues ------------------------------
            # long-lived weight pools first (pool releases are LIFO)
            waop = tc.alloc_tile_pool(name="waop", bufs=1)
            w1p = tc.alloc_tile_pool(name="w1p", bufs=1)
            w2p = tc.alloc_tile_pool(name="w2p", bufs=1)
            xTp = tc.alloc_tile_pool(name="xTp", bufs=1)
            xTn_sb = {}
            for q in QORDER:
                xTn_sb[q] = xTp.tile([128, 4, 2, 512], F8, name=f"xTn{q}")

            ub_sb = const.tile([2, 384], BF)
            rows4 = const.tile([4, S], BF)        # negmu | sd (host)
            r8row = const.tile([1, S], BF)        # rstd/8 at partition 0
            rtmp = const.tile([1, 512], BF)       # bcast staging row
            rstdc = const.tile([128, 16], FP)     # per token-tile rstd col
            trig_sb = const.tile([128, 2, 1024], BF)
            mask_sb = const.tile([128, 384], BF)
            smallc = const.tile([128, 64], FP)
            xsT = const.tile([128, 8, 256], BF)

            # sync queue: host stats rows + small constants
            nc.sync.dma_start(out=rows4[0:2, :], in_=rows2_d)
            nc.sync.dma_start(out=r8row, in_=r8_d)
            nc.sync.dma_start(out=rstdc, in_=rstdc_d)
            nc.sync.dma_start(out=ub_sb, in_=ub_d)
            nc.sync.dma_start(out=trig_sb,
                              in_=trig_d.rearrange("v p t -> p v t"))
            nc.sync.dma_start(out=mask_sb, in_=mask01_d)
            nc.sync.dma_start(out=smallc, in_=smallc_d)
            nc.sync.dma_start(out=xsT, in_=xsT_d)

            # scalar queue: xT chunks + qkv weights (before first sqrt)
            wq_sb = const.tile([128, 3, 4, 2, 128], F8)
            nc.scalar.dma_start(out=xTn_sb[2], in_=xT_d[2].rearrange("j p i t -> p j i t"))
            nc.scalar.dma_start(
                out=wq_sb, in_=wqkv_d)
            nc.scalar.dma_start(out=xTn_sb[3], in_=xT_d[3].rearrange("j p i t -> p j i t"))
            nc.scalar.dma_start(out=xTn_sb[0], in_=xT_d[0].rearrange("j p i t -> p j i t"))
            nc.scalar.dma_start(out=xTn_sb[1], in_=xT_d[1].rearrange("j p i t -> p j i t"))

            gmsa_sb = smallc[:, 0:8]
            gmlp_sb = smallc[:, 8:16]
            b1_sb = smallc[:, 16:48]
            b2_sb = smallc[:, 48:56]
            gb2_sb = smallc[:, 56:64]
            cosT = trig_sb[:, 0, :]
            sinT = trig_sb[:, 1, :]

            ones_sb = const.tile([128, 1], BF)
            nc.vector.memset(ones_sb, 1.0)
            eps128 = const.tile([128, 1], FP)
            nc.vector.memset(eps128, 1e-5)
            eps1 = const.tile([1, 1], FP)
            nc.vector.memset(eps1, 1e-5)
            ident_f = const.tile([128, 128], FP)
            make_identity(nc, ident_f)
            ident_b = const.tile([128, 128], BF)
            nc.vector.tensor_copy(out=ident_b, in_=ident_f)


            qT = qkvr.tile([128, S], BF)
            kT = qkvr.tile([128, S], BF)
            vT = qkvr.tile([128, S], BF)
            qkv_dst = [qT, kT, vT]
            vaug = [vaugp.tile([128, 130], BF, name=f"vaug{kt}")
                    for kt in range(16)]

            onorm = [qkvr.tile([128, N], F8, name=f"onorm{hh}")
                     for hh in range(2)]
            obounce = dram.tile([NCORES, 128, TOK], F8)
            orecvb = dram.tile([NCORES, 128, TOK], F8)
            orecv2 = x2p.tile([128, 8, TOK], F8)
            ounp = tc.alloc_tile_pool(name="ounp", bufs=4)
            o_un = {}
            den4 = [qkvr.tile([128, 512], FP, name=f"den{hh}")
                    for hh in range(2)]
            recip4 = qkvr.tile([128, 512], FP, name="recip4")
            for hh in range(2):
                nc.vector.memset(den4[hh], 1.0)

            # attn_out weights prefetch (gated on gpsimd order below)
            wao_sb = waop.tile([128, 8 * 4 * 2 * 128], F8, name="wao")
            w1_sb = w1p.tile([128, 8 * 4 * 4 * 2 * 128], F8, name="w1")
            w2_sb = w2p.tile([128, 8 * 16 * 2 * 128], F8, name="w2")
            w2r = w2_sb.rearrange("p (m j i c) -> p m j i c", m=8, j=16,
                                  i=2, c=128)
            waor = wao_sb.rearrange("p (m j i c) -> p m j i c",
                                    m=8, j=4, i=2, c=128)
            w1r = w1_sb.rearrange("p (g mi j i c) -> p g mi j i c",
                                  g=8, mi=4, j=4, i=2, c=128)

            qtab = tc.alloc_tile_pool(name="qtab", bufs=2)
            ropep = tc.alloc_tile_pool(name="ropep", bufs=2)
            rbp = tc.alloc_tile_pool(name="rbp", bufs=1)
            qtabs = {}
            mmps = tc.alloc_tile_pool(name="mmps", bufs=2, space="PSUM")
            sps = tc.alloc_tile_pool(name="sps", bufs=2, space="PSUM")
            ops = tc.alloc_tile_pool(name="ops", bufs=2, space="PSUM")

            wqr = wq_sb

            # ---------------- phase builders --------------------------
            def fold(q):
                nsl = slice(512 * q, 512 * q + 512)
                nmod = slice(512 * (q % 2), 512 * (q % 2) + 512)
                rb = rbp.tile([128, 512], BF, tag="rb")
                nc.gpsimd.partition_broadcast(rb, r8row[:, nsl])
                cq = qtab.tile([128, 512], BF, tag="cosq", name=f"cosq{q}")
                sq = qtab.tile([128, 512], BF, tag="sinq", name=f"sinq{q}")
                nc.vector.tensor_mul(cq, cosT[:, nmod], rb)
                nc.vector.tensor_mul(sq, sinT[:, nmod], rb)
                qtabs[q] = (cq, sq)

            def qkv(n):
                nsl = slice(512 * n, 512 * n + 512)
                nmod = slice(512 * (n % 2), 512 * (n % 2) + 512)
                for m in (1, 2, 0):  # q last: more slack for the fold
                    ps = mmps.tile([128, 512], FP, tag="mm", name="qkvps")
                    for j in range(4):
                        nc.tensor.matmul(
                            ps, wqr[:, m, j],
                            xTn_sb[n][:, j], perf_mode=DR,
                            start=(j == 0), stop=False,
                            skip_group_check=True)
                    nc.tensor.matmul(
                        ps, ub_sb[:, 128 * m:128 * m + 128],
                        rows4[0:2, nsl], start=False, stop=True,
                        skip_group_check=True)
                    pb = ropep.tile([128, 512], BF, tag="pb")
                    nc.scalar.copy(out=pb, in_=ps)
                    pbs = ropep.tile([128, 512], BF, tag="pbs")
                    for h in range(2):
                        r = 64 * h
                        nc.vector.tensor_copy(
                            out=pbs[r:r + 32, :], in_=pb[r + 32:r + 64, :])
                        nc.vector.tensor_copy(
                            out=pbs[r + 32:r + 64, :], in_=pb[r:r + 32, :])
                    ca = qtabs[n][0] if m == 0 else cosT[:, nmod]
                    sa = qtabs[n][1] if m == 0 else sinT[:, nmod]
                    t1 = ropep.tile([128, 512], BF, tag="t1")
                    nc.vector.tensor_mul(t1, pb, ca)
                    nc.vector.tensor_mul(pbs, pbs, sa)
                    nc.vector.tensor_add(qkv_dst[m][:, nsl], t1, pbs)

            def vaug_blk(n):
                for kt in range(4 * n, 4 * n + 4):
                    ps = mmps.tile([128, 512], FP, tag="mm", name="vtps")
                    psb = ps.bitcast(BF)
                    nc.tensor.transpose(
                        psb[:, 0:128], vT[:, 128 * kt:128 * kt + 128],
                        ident_b)
                    va = vaug[kt]
                    nc.vector.memset(va[:, 64:65], 1.0)
                    nc.vector.memset(va[:, 129:130], 1.0)
                    # v gets its token's rstd here (per-partition scale)
                    nc.scalar.activation(
                        out=va[:, 0:130].rearrange(
                            "p (h y) -> p h y", y=65)[:, :, 0:64],
                        in_=psb[:, 0:128].rearrange("p (h d) -> p h d", d=64),
                        func=AF.Copy, scale=rstdc[:, kt:kt + 1])

            def attn(c):
                items = sched[c]
                nit = len(items)
                o_ps = {h: ops.tile([65, 512], FP, tag="ops",
                                    name=f"ops{c}_{h}")
                        for h in range(2)}
                q0 = 512 * c
                s_tiles = {}

                def score(idx):
                    kt, c0, c1, mk = items[idx]
                    w = c1 - c0
                    s_ps = sps.tile([128, 2, 512], FP, tag="sps")
                    s_tiles[idx] = s_ps
                    for h in range(2):
                        nc.tensor.matmul(
                            s_ps[:, h, 0:w],
                            kT[64 * h:64 * h + 64,
                               128 * kt:128 * kt + 128],
                            qT[64 * h:64 * h + 64, q0 + c0:q0 + c1],
                            start=True, stop=True,
                            skip_group_check=True)

                def finish(idx):
                    kt, c0, c1, mk = items[idx]
                    w = c1 - c0
                    s_ps = s_tiles.pop(idx)
                    p_sb = ropep.tile([128, 2, 512], BF, tag="pt")
                    sc = rstdc[:, kt:kt + 1]
                    if w == 512:
                        nc.scalar.activation(out=p_sb[:, :, :],
                                             in_=s_ps[:, :, :],
                                             func=AF.Exp, scale=sc)
                    else:
                        for h in range(2):
                            nc.scalar.activation(
                                out=p_sb[:, h, 0:w],
                                in_=s_ps[:, h, 0:w],
                                func=AF.Exp, scale=sc)
                    if mk is not None:
                        mo = MASK_OFF[mk]
                        for h in range(2):
                            nc.vector.tensor_mul(
                                p_sb[:, h, 0:128], p_sb[:, h, 0:128],
                                mask_sb[:, mo:mo + 128])
                    for h in range(2):
                        nc.tensor.matmul(
                            o_ps[h][:, c0:c1],
                            vaug[kt][:, 65 * h:65 * h + 65],
                            p_sb[:, h, 0:w], start=(idx == 0),
                            stop=(idx == nit - 1),
                            skip_group_check=True)

                score(0)
                for idx in range(1, nit):
                    score(idx)
                    finish(idx - 1)
                finish(nit - 1)
                hh = 1 if c >= 2 else 0
                for h in range(2):
                    k = 2 * c + h
                    r = 32 * (k % 4)
                    o_un[k] = ounp.tile([64, 512], BF, tag="oun",
                                        name=f"oun{k}")
                    nc.scalar.copy(out=o_un[k], in_=o_ps[h][0:64, :])
                    nc.scalar.copy(out=den4[hh][r:r + 1, :],
                                   in_=o_ps[h][64:65, :])

            def norm_half(hh):
                # streams of this half: chunks (2,3) for hh=1, (0,1) for 0
                nc.vector.reciprocal_approx_fast(
                    out=recip4, in_=den4[hh])
                for c in ((2, 3) if hh else (0, 1)):
                    for h in range(2):
                        k = 2 * c + h
                        r = 32 * (k % 4)
                        rbc = ropep.tile([128, 512], BF, tag="t1",
                                         name="rbc")
                        nc.scalar.copy(out=rtmp, in_=recip4[r:r + 1, :])
                        nc.gpsimd.partition_broadcast(rbc[0:64, :], rtmp)
                        nc.vector.tensor_mul(
                            onorm[hh][64 * h:64 * h + 64,
                                      (512 * c) % N:(512 * c) % N + 512],
                            o_un[k], rbc[0:64, :])
                nc.sync.dma_start(
                    out=obounce[4 * hh:4 * hh + 4].rearrange(
                        "j p t -> p j t"),
                    in_=onorm[hh].rearrange("p (j t) -> p j t", t=TOK))

            # ---------------- emission order --------------------------
            fold(2)
            fold(3)
            qkv(2)
            vaug_blk(2)
            qkv(3)
            vaug_blk(3)
            fold(0)
            fold(1)
            attn(2)
            # gpsimd order gate: weight prefetch waits for the last x
            # chunk's arrival, then streams under the attention window
            gate = rbp.tile([1, 1], F8, tag="gate")
            nc.gpsimd.tensor_copy(out=gate, in_=xTn_sb[1][0:1, 0, 0:1, 0])
            nc.gpsimd.dma_start(out=wao_sb.rearrange(
                "p (m j i c) -> p m j i c", m=8, j=4, i=2, c=128),
                in_=wao_d)
            nc.gpsimd.dma_start(out=w1_sb.rearrange(
                "p (g mi j i c) -> p g mi j i c", g=8, mi=4, j=4, i=2,
                c=128), in_=w1_d)
            nc.gpsimd.dma_start(out=w2_sb.rearrange(
                "p (m j i c) -> p m j i c", m=8, j=16, i=2, c=128),
                in_=w2_d)
            qkv(0)
            vaug_blk(0)
            attn(3)
            norm_half(1)
            qkv(1)
            vaug_blk(1)
            attn(0)
            attn(1)
            norm_half(0)

            if dbg_d is not None:
                nc.sync.dma_start(out=dbg_d[0], in_=qT)
                nc.sync.dma_start(out=dbg_d[1], in_=kT)
                nc.sync.dma_start(out=dbg_d[2], in_=vT)
                for hh in range(2):
                    nc.sync.dma_start(out=dbgf8_d[0][:, N * hh:N * hh + N],
                                      in_=onorm[hh])
                    nc.sync.dma_start(
                        out=dbg32_d[0][:, 512 * hh:512 * hh + 512],
                        in_=den4[hh])
                nc.sync.dma_start(out=dbg32_d[0][:, 1024:1040], in_=rstdc)
                nc.sync.dma_start(out=dbg_d[5][0:4, :], in_=rows4)
                nc.sync.dma_start(out=dbg_d[6][0:1, :], in_=r8row)

            if single:
                nc.sync.dma_start(out=orecvb[:], in_=obounce[:])
            else:
                nc.gpsimd.collective_compute(
                    "AllToAll", ALU.bypass,
                    replica_groups=[list(range(NCORES))],
                    ins=[obounce.opt()], outs=[orecvb.opt()])
            nc.sync.dma_start(
                out=orecv2, in_=orecvb.rearrange("k r t -> r k t"))

            ops.release()
            sps.release()
            mmps.release()
            rbp.release()
            ropep.release()
            qtab.release()
            ounp.release()
            xTp.release()

            # ---------------- phase 4: attn_out + residual + LN2 stats -
            x2T = [x2p.tile([128, TOK], FP, name=f"x2T{m}") for m in range(8)]
            x2b = [x2p.tile([128, TOK], BF, name=f"x2b{m}") for m in range(8)]
            sqb = [x2p.tile([128, TOK], BF, name=f"sqb{m}") for m in range(8)]
            h2dr = [x2p.tile([128, 2, TOK], F8, name=f"h2dr{j}")
                    for j in range(4)]
            with tc.tile_pool(name="aops", bufs=3, space="PSUM") as aops, \
                 tc.tile_pool(name="l2ps", bufs=1, space="PSUM") as l2ps, \
                 tc.tile_pool(name="aot", bufs=3) as aot, \
                 tc.tile_pool(name="l2t", bufs=1) as l2t:
                sum_ps = l2ps.tile([1, TOK], FP, tag="l2sum")
                ssq_ps = l2ps.tile([1, TOK], FP, tag="l2ssq")

                def ao_chain(m):
                    ps = aops.tile([128, TOK], FP, tag="aops")
                    for j in range(4):
                        nc.tensor.matmul(
                            ps, waor[:, m, j],
                            orecv2[:, 2 * j:2 * j + 2, :], perf_mode=DR,
                            start=(j == 0), stop=(j == 3),
                            skip_group_check=True)
                    ao_sb = aot.tile([128, TOK], FP, tag="ao")
                    nc.scalar.copy(out=ao_sb, in_=ps)
                    nc.vector.scalar_tensor_tensor(
                        out=x2T[m], in0=ao_sb,
                        scalar=gmsa_sb[:, m:m + 1],
                        in1=xsT[:, m, :], op0=ALU.mult, op1=ALU.add)
                    nc.vector.tensor_copy(out=x2b[m], in_=x2T[m])
                    nc.vector.tensor_mul(sqb[m], x2b[m], x2b[m])

                def ln2_acc(m):
                    nc.tensor.matmul(sum_ps, ones_sb, x2b[m],
                                     start=(m == 0), stop=(m == 7),
                                     skip_group_check=True)
                    nc.tensor.matmul(ssq_ps, ones_sb, sqb[m],
                                     start=(m == 0), stop=(m == 7),
                                     skip_group_check=True)

                ao_chain(0)
                for m in range(1, 8):
                    ao_chain(m)
                    ln2_acc(m - 1)
                ln2_acc(7)

                mu2f = l2t.tile([1, TOK], FP)
                nc.vector.tensor_scalar_mul(mu2f, sum_ps, 1.0 / D)
                var2 = l2t.tile([1, TOK], FP)
                musq = l2t.tile([1, TOK], FP)
                nc.vector.tensor_mul(musq, mu2f, mu2f)
                nc.vector.tensor_scalar_mul(var2, ssq_ps, 1.0 / D)
                nc.vector.tensor_sub(var2, var2, musq)
                sd2 = l2t.tile([1, TOK], FP)
                nc.scalar.activation(out=sd2, in_=var2, func=AF.Sqrt,
                                     bias=eps1, scale=1.0)
                rstd2 = l2t.tile([1, TOK], FP)
                nc.vector.reciprocal_approx_fast(out=rstd2, in_=sd2)
                # row broadcasts (sources live at partition 0)
                mu2bc = l2t.tile([128, TOK], FP)
                nc.gpsimd.partition_broadcast(mu2bc, mu2f)
                rstd2bc = l2t.tile([128, TOK], FP)
                nc.gpsimd.partition_broadcast(rstd2bc, rstd2)
                for k in range(8):
                    u = l2t.tile([128, TOK], FP, tag="u", bufs=2)
                    nc.vector.tensor_sub(u, x2T[k], mu2bc)
                    nc.vector.tensor_mul(h2dr[k // 2][:, k % 2, :],
                                         u, rstd2bc)

            if dbg_d is not None:
                nc.sync.dma_start(
                    out=dbgf8_d[1].rearrange("p (k t) -> p k t", t=TOK),
                    in_=orecv2)
                for m in range(8):
                    nc.sync.dma_start(
                        out=dbg32_d[1][:, TOK * m:TOK * m + TOK],
                        in_=x2T[m])
                for j in range(4):
                    nc.sync.dma_start(
                        out=dbgf8_d[2].rearrange(
                            "p (j i t) -> p j i t", j=4, i=2)[:, j],
                        in_=h2dr[j])

            # ---------------- phase 6: MLP (fp8 DoubleRow) -------------
            g_dr = gp.tile([128, 16, 2, TOK], F8, name="g_dr")
            with tc.tile_pool(name="m1ps", bufs=3, space="PSUM") as m1ps:
                for g in range(8):
                    for mi in range(4):
                        m = 4 * g + mi
                        ps = m1ps.tile([128, TOK], FP, tag="m1")
                        for j in range(4):
                            nc.tensor.matmul(ps, w1r[:, g, mi, j],
                                             h2dr[j], perf_mode=DR,
                                             start=(j == 0), stop=(j == 3))
                        gfunc = (AF.Identity if os.environ.get("DBG_NO_GELU")
                                 else AF.Gelu_apprx_tanh)
                        nc.scalar.activation(out=g_dr[:, m // 2, m % 2, :],
                                             in_=ps, func=gfunc,
                                             bias=b1_sb[:, m:m + 1],
                                             scale=1.0 / W1SCALE)

            outT = gp.tile([128, 8, TOK], FP, name="outT")
            with tc.tile_pool(name="m2ps", bufs=3, space="PSUM") as m2ps, \
                 tc.tile_pool(name="outp", bufs=3) as outp:
                for m in range(8):
                    ps = m2ps.tile([128, TOK], FP, tag="m2")
                    for j in range(16):
                        nc.tensor.matmul(ps, w2r[:, m, j], g_dr[:, j],
                                         perf_mode=DR,
                                         start=(j == 0), stop=(j == 15))
                    # psum = W2SCALE*(m - b2); evac: gmlp/W2SCALE * ps + gb2
                    mo = outp.tile([128, TOK], FP, tag="mo")
                    nc.scalar.activation(out=mo, in_=ps, func=AF.Identity,
                                         bias=gb2_sb[:, m:m + 1],
                                         scale=gmlp_sb[:, m:m + 1])
                    nc.vector.tensor_add(outT[:, m, :], mo, x2T[m])
                    nc.sync.dma_start(out=out_d[:, m, :],
                                      in_=outT[:, m, :])
            w2p.release()
            w1p.release()
            waop.release()

    nc.compile()
    return nc


# ---------------------------------------------------------------------------
# host side
# ---------------------------------------------------------------------------

_NC = None


def _get_nc():
    global _NC
    if _NC is None:
        _NC = build_program()
    return _NC


def _mask01_tiles():
    a = np.arange(128) // BS
    diag = (a[:, None] == a[None, :])
    strict = (a[None, :] > a[:, None])
    incl = (a[None, :] >= a[:, None])
    m = np.concatenate([diag, strict, incl],
                       axis=1).astype(np.float32)
    return np.ascontiguousarray(m.astype(bf16))


def _tile4(wT, km, mm):
    """[K, M] -> (m, p, k, c) with arr[m, p, k, c] = wT[128k+p, 128m+c]."""
    return wT.reshape(km, 128, mm, 128).transpose(2, 1, 0, 3)


def _prep_inputs(x, c, cos, sin, norm1_w, qkv_w, attn_out_w, norm2_w,
                 mlp_w1, mlp_b1, mlp_w2, mlp_b2, adaLN_w, adaLN_b):
    f32 = np.float32
    x = np.asarray(x, f32).reshape(S, D)
    c = np.asarray(c, f32).reshape(COND)
    cos = np.asarray(cos, f32)
    sin = np.asarray(sin, f32)
    qkv_w = np.asarray(qkv_w, f32)
    mlp_w1 = np.asarray(mlp_w1, f32)

    # adaLN modulation on host
    mods = adaLN_w.astype(f32) @ c + np.asarray(adaLN_b, f32)
    sh_msa, sc_msa, g_msa, sh_mlp, sc_mlp, g_mlp = mods.reshape(6, D)

    gam1 = (1.0 + sc_msa) * np.asarray(norm1_w, f32)          # [D]
    qkv_ws = qkv_w * gam1[None, :]                            # [3D, D]
    u_qkv = qkv_ws.sum(axis=1)                                # [3D]
    b_qkv = qkv_w @ sh_msa                                    # [3D]

    gam2 = (1.0 + sc_mlp) * np.asarray(norm2_w, f32)          # [D]
    w1s = mlp_w1 * gam2[None, :]                              # [4D, D]
    b1f = np.asarray(mlp_b1, f32) + mlp_w1 @ sh_mlp           # [4D]
    b2 = np.asarray(mlp_b2, f32)

    xb = x.astype(bf16)
    # LN1 stats on host (fp32)
    mu = x.mean(axis=1)
    sd = np.sqrt(x.var(axis=1) + 1e-5)
    rstd = 1.0 / sd
    rows2 = np.ascontiguousarray(np.stack([-mu, sd]).astype(bf16))
    r8row = np.ascontiguousarray((rstd * 0.125)[None, :].astype(bf16))
    rstdc = np.ascontiguousarray(rstd.reshape(16, 128).T.astype(f32))
    # qkv moving: (n, j, p, i, t) fp8
    xT8 = np.ascontiguousarray(
        x.T.reshape(4, 2, 128, 4, 512).transpose(3, 0, 2, 1, 4)
        .astype(fp8))
    xTb = xb.T.reshape(8, 128, S)                             # (k, p, t)

    # rope tables [2, 128, 1024]: cos | dest-signed sin; 1/WQSCALE folded.
    cs = np.concatenate([cos, cos], axis=-1).T                # [64, N]
    ss = np.concatenate([-sin.T, sin.T], axis=0)              # [64, N]
    cos128 = np.vstack([cs, cs])                              # [128, N]
    sin128 = np.vstack([ss, ss])
    trig = np.ascontiguousarray(
        (np.stack([cos128, sin128]) / WQSCALE).astype(bf16))

    # attn_out: (p=(i,hd), m, j, i2, c); f = (2*(2j+i2)+i)*64 + hd
    waoT = np.ascontiguousarray(
        np.clip(np.asarray(attn_out_w, f32).T * WAOSCALE, -240, 240)
        .reshape(4, 2, 2, 64, 8, 128).transpose(2, 3, 4, 0, 1, 5)
        .reshape(128, 8, 4, 2, 128).astype(fp8))
    # w1: (p, g, mi, j, i, c)
    w1q = np.clip(_tile4(w1s.T, 8, 32) * W1SCALE, -240, 240)  # [32,128,8,128]
    w1T = np.ascontiguousarray(
        w1q.reshape(8, 4, 128, 4, 2, 128).transpose(2, 0, 1, 3, 4, 5)
        .astype(fp8))                                 # (p, g, mi, j, i, c)
    # w2: (p, m, j, i, c)
    w2q = np.clip(_tile4(np.asarray(mlp_w2, f32).T, 32, 8) * W2SCALE,
                  -240, 240)                          # [8, 128, 32, 128]
    w2T = np.ascontiguousarray(
        w2q.reshape(8, 128, 16, 2, 128).transpose(1, 0, 2, 3, 4)
        .astype(fp8))

    smallc = np.ascontiguousarray(np.hstack([
        (g_msa / WAOSCALE).reshape(8, 128).T,
        (g_mlp / W2SCALE).reshape(8, 128).T,
        b1f.reshape(32, 128).T,
        b2.reshape(8, 128).T,
        (g_mlp * b2).reshape(8, 128).T]).astype(f32))         # [128, 64]

    common = {
        "rows2": rows2, "r8row": r8row, "rstdc": rstdc,
        "xT": xT8,
        "waoT": waoT, "w1T": w1T, "w2T": w2T,
        "smallc": smallc, "trig": trig,
        "mask01": _mask01_tiles(),
    }
    in_maps = []
    for j in range(NCORES):
        wq = np.stack([
            np.clip(
                qkv_ws[s * D + 128 * j: s * D + 128 * j + 128].T
                * WQSCALE, -240, 240)
            .reshape(4, 2, 128, 128).transpose(0, 2, 1, 3)
            for s in range(3)])  # [s, j, p, i, c]
        wq = np.ascontiguousarray(
            wq.transpose(2, 0, 1, 3, 4).astype(fp8))  # (p, s, j, i, c)
        ub = np.stack([
            np.concatenate([u_qkv[s * D + 128 * j: s * D + 128 * j + 128]
                            for s in range(3)]),
            np.concatenate([b_qkv[s * D + 128 * j: s * D + 128 * j + 128]
                            for s in range(3)])]) * WQSCALE  # [2, 384]
        m = dict(common)
        m["wqkvT"] = wq
        m["ubrow"] = np.ascontiguousarray(ub.astype(bf16))
        m["xsliceT"] = np.ascontiguousarray(
            xTb[:, :, TOK * j:TOK * j + TOK].transpose(1, 0, 2))  # (p,k,t)
        in_maps.append(m)
    return in_maps


def _assemble(res):
    """Gather per-core outputs [128, 8, TOK] (p, m, t) into [1, S, D]."""
    parts = []
    for j in range(NCORES):
        o = res.results[j]["out"]  # [128, 8, TOK]
        parts.append(np.ascontiguousarray(
            o.transpose(2, 1, 0).reshape(TOK, D)))
    return np.concatenate(parts, axis=0).reshape(1, S, D).astype(np.float32)


def kernel(**inputs):
    nc = _get_nc()
    in_maps = _prep_inputs(**inputs)
    res = run_bass_kernel_spmd(nc, in_maps, core_ids=list(range(NCORES)))
    return _assemble(res)


# revision 63
# speedup vs baseline: 1.4438x; 1.0810x over previous
"""DiT block with block-diffusion sparse attention on 8 Trainium2 NeuronCores.

v4 strategy (evolution of v3; see kernel_v3.py):
  - Clean-half-first software pipeline: per-512-token-quarter LN1 stats
    (bn_stats on fp8 token-major x) feed QKV chunks as they complete; the
    clean half (tokens 1024:2048) goes first so attention chunks c=2,3 can
    run interleaved with the noisy half's QKV matmuls on the PE.
  - rstd is NOT folded into the rope tables for k/v. Instead:
      k: per-partition scale at the softmax EXP (scores are [k, q] so
         rstd_k is a per-partition column; q's rstd*0.125 is folded into
         the q rope tables via one row-broadcast per quarter).
      v: per-partition scale on the V-transpose PSUM evacuation.
    This removes 2/3 of the rstd broadcast/fold work.
  - Stats avoid the DRAM bounce: per-tile [128,4] (negmu, sd, rstd/8) PE
    transposes into rows4[4, S]; rstd also kept token-tile-major in
    rstdc[128,16] for the exp/v scales.
  - rotate-half swap copies on GpSimd; attention mask multiplies on DVE
    (idle during attention); softmax reciprocal via reciprocal_approx_fast.
  - Softmax denominators/normalization per half: the clean half is
    normalized and staged to DRAM while the noisy half's attention still
    runs; only the noisy half's norm is exposed before the AllToAll.
  - Single consolidated DMAs with >=1KB descriptor lines; issues spread
    over the sync/tensor/gpsimd queues; w1/w2 prefetch gated behind the
    x loads via gpsimd program order.
  - attn_out uses full 128-partition stationary tiles (2 source cores per
    DoubleRow pair) - 2x fewer PE cycles than v3's 64-partition layout.
  - LN2 sum/sumsq matmuls interleaved with the attn_out chains; LN2 row
    broadcasts via stride-0 DMA instead of PE rank-1 + ACT evac.
"""

import os
import numpy as np
import ml_dtypes

import concourse.bass as bass
import concourse.tile as tile
from concourse import bacc, mybir
from concourse.bass_utils import run_bass_kernel_spmd
from concourse.masks import make_identity

bf16 = ml_dtypes.bfloat16
fp8 = ml_dtypes.float8_e4m3
FP = mybir.dt.float32
BF = mybir.dt.bfloat16
F8 = mybir.dt.float8e4
AF = mybir.ActivationFunctionType
ALU = mybir.AluOpType
DR = mybir.MatmulPerfMode.DoubleRow
WQSCALE = 64.0
WAOSCALE = 64.0
W1SCALE = 64.0
W2SCALE = 128.0

NCORES = 8
S, N, D, H, HD, BS, COND = 2048, 1024, 1024, 16, 64, 16, 128
TOK = S // NCORES  # 256 tokens per core after A2A

QORDER = [2, 3, 0, 1]  # clean half first


def _attn_schedule():
    """Per q-chunk list of (ktile, col0, col1, mask) in S^T orientation."""
    sched = []
    for c in range(4):
        items = []
        if c < 2:  # noisy q chunk
            for j in range(4 * c + 4):  # clean k tiles, bq > bk
                js = j - 4 * c
                if js < 0:
                    items.append((8 + j, 0, 512, None))
                else:
                    items.append((8 + j, 128 * js, 512, "strict"))
            for s in range(4):  # own-block diagonal (noisy k)
                items.append((4 * c + s, 128 * s, 128 * s + 128, "diag"))
        else:  # clean q chunk, bq >= bk
            cq = c - 2
            for j in range(4 * cq + 4):
                js = j - 4 * cq
                if js < 0:
                    items.append((8 + j, 0, 512, None))
                else:
                    items.append((8 + j, 128 * js, 512, "incl"))
        assert items[0][1] == 0 and items[0][2] == 512
        sched.append(items)
    return sched


MASK_OFF = {"diag": 0, "strict": 128, "incl": 256}


def build_program(single=False, dbg=False):
    nc = bacc.Bacc(
        "TRN2", target_bir_lowering=False, debug=False,
        enable_asserts=False, num_devices=1 if single else NCORES,
    )

    def din(name, shape, dt=BF):
        return nc.dram_tensor(name, shape, dt, kind="ExternalInput").ap()

    rows2_d = din("rows2", [2, S])                    # negmu | sd (host)
    r8_d = din("r8row", [1, S])                       # rstd/8 (host)
    rstdc_d = din("rstdc", [128, 16], FP)             # rstd token-tile cols
    xT_d = din("xT", [4, 4, 128, 2, 512], F8)         # (n, j, p, i, t)
    trig_d = din("trig", [2, 128, 1024])              # cos128 | sin128(signed)
    mask01_d = din("mask01", [128, 384])              # diag|strict|incl
    wqkv_d = din("wqkvT", [128, 3, 4, 2, 128], F8)    # (p, s, j, i, c) scaled
    ub_d = din("ubrow", [2, 384])                     # (u; b) per-core slice
    xsT_d = din("xsliceT", [128, 8, 256])             # residual (p, k, t)
    wao_d = din("waoT", [128, 8, 4, 2, 128], F8)      # (p=(i,hd), m, j, i2, c)
    w1_d = din("w1T", [128, 8, 4, 4, 2, 128], F8)     # (p, g, mi, j, i, c)
    w2_d = din("w2T", [128, 8, 16, 2, 128], F8)       # (p, m, j, i, c)
    smallc_d = din("smallc", [128, 64], FP)           # gmsa|gmlp|b1'|b2|gb2
    out_d = nc.dram_tensor("out", [128, 8, TOK], FP, kind="ExternalOutput").ap()
    dbg_d = (nc.dram_tensor("dbg", [8, 128, S], BF,
                            kind="ExternalOutput").ap() if dbg else None)
    dbgf8_d = (nc.dram_tensor("dbgf8", [3, 128, S], F8,
                              kind="ExternalOutput").ap() if dbg else None)
    dbg32_d = (nc.dram_tensor("dbg32", [2, 128, S], FP,
                              kind="ExternalOutput").ap() if dbg else None)

    sched = _attn_schedule()

    with tile.TileContext(nc) as tc:
        with tc.tile_pool(name="const", bufs=1) as const, \
             tc.tile_pool(name="dram", bufs=1, space="DRAM") as dram, \
             tc.tile_pool(name="qkvr", bufs=1) as qkvr, \
             tc.tile_pool(name="vaugp", bufs=1) as vaugp, \
             tc.tile_pool(name="x2p", bufs=1) as x2p, \
             tc.tile_pool(name="gp", bufs=1) as gp:

            # ---------------- DMA issues ------------------------------
            # long-lived weight pools first (pool releases are LIFO)
            waop = tc.alloc_tile_pool(name="waop", bufs=1)
            w1p = tc.alloc_tile_pool(name="w1p", bufs=1)
            w2p = tc.alloc_tile_pool(name="w2p", bufs=1)
            xTp = tc.alloc_tile_pool(name="xTp", bufs=1)
            xTn_sb = {}
            for q in QORDER:
                xTn_sb[q] = xTp.tile([128, 4, 2, 512], F8, name=f"xTn{q}")

            ub_sb = const.tile([2, 384], BF)
            rows4 = const.tile([4, S], BF)        # negmu | sd (host)
            r8row = const.tile([1, S], BF)        # rstd/8 at partition 0
            rtmp = const.tile([1, 512], BF)       # bcast staging row
            rstdc = const.tile([128, 16], FP)     # per token-tile rstd col
            trig_sb = const.tile([128, 2, 1024], BF)
            mask_sb = const.tile([128, 384], BF)
            smallc = const.tile([128, 64], FP)
            xsT = const.tile([128, 8, 256], BF)

            # sync queue: host stats rows + small constants
            nc.sync.dma_start(out=rows4[0:2, :], in_=rows2_d)
            nc.sync.dma_start(out=r8row, in_=r8_d)
            nc.sync.dma_start(out=rstdc, in_=rstdc_d)
            nc.sync.dma_start(out=ub_sb, in_=ub_d)
            nc.sync.dma_start(out=trig_sb,
                              in_=trig_d.rearrange("v p t -> p v t"))
            nc.sync.dma_start(out=mask_sb, in_=mask01_d)
            nc.sync.dma_start(out=smallc, in_=smallc_d)
            nc.sync.dma_start(out=xsT, in_=xsT_d)

            # scalar queue: xT chunks + qkv weights (before first sqrt)
            wq_sb = const.tile([128, 3, 4, 2, 128], F8)
            nc.scalar.dma_start(out=xTn_sb[2], in_=xT_d[2].rearrange("j p i t -> p j i t"))
            nc.scalar.dma_start(
                out=wq_sb, in_=wqkv_d)
            nc.scalar.dma_start(out=xTn_sb[3], in_=xT_d[3].rearrange("j p i t -> p j i t"))
            nc.scalar.dma_start(out=xTn_sb[0], in_=xT_d[0].rearrange("j p i t -> p j i t"))
            nc.scalar.dma_start(out=xTn_sb[1], in_=xT_d[1].rearrange("j p i t -> p j i t"))

            gmsa_sb = smallc[:, 0:8]
            gmlp_sb = smallc[:, 8:16]
            b1_sb = smallc[:, 16:48]
            b2_sb = smallc[:, 48:56]
            gb2_sb = smallc[:, 56:64]
            cosT = trig_sb[:, 0, :]
            sinT = trig_sb[:, 1, :]

            ones_sb = const.tile([128, 1], BF)
            nc.vector.memset(ones_sb, 1.0)
            eps128 = const.tile([128, 1], FP)
            nc.vector.memset(eps128, 1e-5)
            eps1 = const.tile([1, 1], FP)
            nc.vector.memset(eps1, 1e-5)
            ident_f = const.tile([128, 128], FP)
            make_identity(nc, ident_f)
            ident_b = const.tile([128, 128], BF)
            nc.vector.tensor_copy(out=ident_b, in_=ident_f)


            qT = qkvr.tile([128, S], BF)
            kT = qkvr.tile([128, S], BF)
            vT = qkvr.tile([128, S], BF)
            qkv_dst = [qT, kT, vT]
            vaug = [vaugp.tile([128, 130], BF, name=f"vaug{kt}")
                    for kt in range(16)]

            onorm = [qkvr.tile([128, N], F8, name=f"onorm{hh}")
                     for hh in range(2)]
            obounce = dram.tile([NCORES, 128, TOK], F8)
            orecvb = dram.tile([NCORES, 128, TOK], F8)
            orecv2 = x2p.tile([128, 8, TOK], F8)
            ounp = tc.alloc_tile_pool(name="ounp", bufs=4)
            o_un = {}
            den4 = [qkvr.tile([128, 512], FP, name=f"den{hh}")
                    for hh in range(2)]
            recip4 = qkvr.tile([128, 512], FP, name="recip4")
            for hh in range(2):
                nc.vector.memset(den4[hh], 1.0)

            # attn_out weights prefetch (gated on gpsimd order below)
            wao_sb = waop.tile([128, 8 * 4 * 2 * 128], F8, name="wao")
            w1_sb = w1p.tile([128, 8 * 4 * 4 * 2 * 128], F8, name="w1")
            w2_sb = w2p.tile([128, 8 * 16 * 2 * 128], F8, name="w2")
            w2r = w2_sb.rearrange("p (m j i c) -> p m j i c", m=8, j=16,
                                  i=2, c=128)
            waor = wao_sb.rearrange("p (m j i c) -> p m j i c",
                                    m=8, j=4, i=2, c=128)
            w1r = w1_sb.rearrange("p (g mi j i c) -> p g mi j i c",
                                  g=8, mi=4, j=4, i=2, c=128)

            qtab = tc.alloc_tile_pool(name="qtab", bufs=2)
            ropep = tc.alloc_tile_pool(name="ropep", bufs=2)
            rbp = tc.alloc_tile_pool(name="rbp", bufs=1)
            nrmp = tc.alloc_tile_pool(name="nrmp", bufs=2)
            qtabs = {}
            mmps = tc.alloc_tile_pool(name="mmps", bufs=2, space="PSUM")
            sps = tc.alloc_tile_pool(name="sps", bufs=2, space="PSUM")
            ops = tc.alloc_tile_pool(name="ops", bufs=2, space="PSUM")

            wqr = wq_sb

            # ---------------- phase builders --------------------------
            def fold(q):
                nsl = slice(512 * q, 512 * q + 512)
                nmod = slice(512 * (q % 2), 512 * (q % 2) + 512)
                rb = rbp.tile([128, 512], BF, tag="rb")
                nc.gpsimd.partition_broadcast(rb, r8row[:, nsl])
                cq = qtab.tile([128, 512], BF, tag="cosq", name=f"cosq{q}")
                sq = qtab.tile([128, 512], BF, tag="sinq", name=f"sinq{q}")
                nc.vector.tensor_mul(cq, cosT[:, nmod], rb)
                nc.vector.tensor_mul(sq, sinT[:, nmod], rb)
                qtabs[q] = (cq, sq)

            def qkv(n):
                nsl = slice(512 * n, 512 * n + 512)
                nmod = slice(512 * (n % 2), 512 * (n % 2) + 512)
                for m in (1, 2, 0):  # q last: more slack for the fold
                    ps = mmps.tile([128, 512], FP, tag="mm", name="qkvps")
                    for j in range(4):
                        nc.tensor.matmul(
                            ps, wqr[:, m, j],
                            xTn_sb[n][:, j], perf_mode=DR,
                            start=(j == 0), stop=False,
                            skip_group_check=True)
                    nc.tensor.matmul(
                        ps, ub_sb[:, 128 * m:128 * m + 128],
                        rows4[0:2, nsl], start=False, stop=True,
                        skip_group_check=True)
                    pb = ropep.tile([128, 512], BF, tag="pb")
                    nc.scalar.copy(out=pb, in_=ps)
                    pbs = ropep.tile([128, 512], BF, tag="pbs")
                    for h in range(2):
                        r = 64 * h
                        nc.vector.tensor_copy(
                            out=pbs[r:r + 32, :], in_=pb[r + 32:r + 64, :])
                        nc.vector.tensor_copy(
                            out=pbs[r + 32:r + 64, :], in_=pb[r:r + 32, :])
                    ca = qtabs[n][0] if m == 0 else cosT[:, nmod]
                    sa = qtabs[n][1] if m == 0 else sinT[:, nmod]
                    t1 = ropep.tile([128, 512], BF, tag="t1")
                    nc.vector.tensor_mul(t1, pb, ca)
                    nc.vector.tensor_mul(pbs, pbs, sa)
                    nc.vector.tensor_add(qkv_dst[m][:, nsl], t1, pbs)

            def vaug_blk(n):
                for kt in range(4 * n, 4 * n + 4):
                    ps = mmps.tile([128, 512], FP, tag="mm", name="vtps")
                    psb = ps.bitcast(BF)
                    nc.tensor.transpose(
                        psb[:, 0:128], vT[:, 128 * kt:128 * kt + 128],
                        ident_b)
                    va = vaug[kt]
                    nc.vector.memset(va[:, 64:65], 1.0)
                    nc.vector.memset(va[:, 129:130], 1.0)
                    # v gets its token's rstd here (per-partition scale)
                    nc.scalar.activation(
                        out=va[:, 0:130].rearrange(
                            "p (h y) -> p h y", y=65)[:, :, 0:64],
                        in_=psb[:, 0:128].rearrange("p (h d) -> p h d", d=64),
                        func=AF.Copy, scale=rstdc[:, kt:kt + 1])

            def attn(c):
                items = sched[c]
                nit = len(items)
                o_ps = {h: ops.tile([65, 512], FP, tag="ops",
                                    name=f"ops{c}_{h}")
                        for h in range(2)}
                q0 = 512 * c
                s_tiles = {}

                def score(idx):
                    kt, c0, c1, mk = items[idx]
                    w = c1 - c0
                    s_ps = sps.tile([128, 2, 512], FP, tag="sps")
                    s_tiles[idx] = s_ps
                    for h in range(2):
                        nc.tensor.matmul(
                            s_ps[:, h, 0:w],
                            kT[64 * h:64 * h + 64,
                               128 * kt:128 * kt + 128],
                            qT[64 * h:64 * h + 64, q0 + c0:q0 + c1],
                            start=True, stop=True,
                            skip_group_check=True)

                def finish(idx):
                    kt, c0, c1, mk = items[idx]
                    w = c1 - c0
                    s_ps = s_tiles.pop(idx)
                    p_sb = ropep.tile([128, 2, 512], BF, tag="pt")
                    sc = rstdc[:, kt:kt + 1]
                    if w == 512:
                        nc.scalar.activation(out=p_sb[:, :, :],
                                             in_=s_ps[:, :, :],
                                             func=AF.Exp, scale=sc)
                    else:
                        for h in range(2):
                            nc.scalar.activation(
                                out=p_sb[:, h, 0:w],
                                in_=s_ps[:, h, 0:w],
                                func=AF.Exp, scale=sc)
                    if mk is not None:
                        mo = MASK_OFF[mk]
                        for h in range(2):
                            nc.vector.tensor_mul(
                                p_sb[:, h, 0:128], p_sb[:, h, 0:128],
                                mask_sb[:, mo:mo + 128])
                    for h in range(2):
                        nc.tensor.matmul(
                            o_ps[h][:, c0:c1],
                            vaug[kt][:, 65 * h:65 * h + 65],
                            p_sb[:, h, 0:w], start=(idx == 0),
                            stop=(idx == nit - 1),
                            skip_group_check=True)

                score(0)
                for idx in range(1, nit):
                    score(idx)
                    finish(idx - 1)
                finish(nit - 1)
                hh = 1 if c >= 2 else 0
                for h in range(2):
                    k = 2 * c + h
                    r = 32 * (k % 4)
                    o_un[k] = ounp.tile([64, 512], BF, tag="oun",
                                        name=f"oun{k}")
                    nc.scalar.copy(out=o_un[k], in_=o_ps[h][0:64, :])
                    nc.scalar.copy(out=den4[hh][r:r + 1, :],
                                   in_=o_ps[h][64:65, :])

            def norm_half(hh):
                # streams of this half: chunks (2,3) for hh=1, (0,1) for 0
                nc.vector.reciprocal_approx_fast(
                    out=recip4, in_=den4[hh])
                for c in ((2, 3) if hh else (0, 1)):
                    for h in range(2):
                        k = 2 * c + h
                        r = 32 * (k % 4)
                        rt = nrmp.tile([1, 512], BF, tag="rt")
                        nc.scalar.copy(out=rt, in_=recip4[r:r + 1, :])
                        rbc = nrmp.tile([64, 512], BF, tag="rbc")
                        nc.gpsimd.partition_broadcast(rbc, rt)
                        nc.vector.tensor_mul(
                            onorm[hh][64 * h:64 * h + 64,
                                      (512 * c) % N:(512 * c) % N + 512],
                            o_un[k], rbc)
                nc.sync.dma_start(
                    out=obounce[4 * hh:4 * hh + 4].rearrange(
                        "j p t -> p j t"),
                    in_=onorm[hh].rearrange("p (j t) -> p j t", t=TOK))

            # ---------------- emission order --------------------------
            fold(2)
            fold(3)
            qkv(2)
            vaug_blk(2)
            qkv(3)
            vaug_blk(3)
            fold(0)
            fold(1)
            attn(2)
            # gpsimd order gate: weight prefetch waits for the last x
            # chunk's arrival, then streams under the attention window
            gate = rbp.tile([1, 1], F8, tag="gate")
            nc.gpsimd.tensor_copy(out=gate, in_=xTn_sb[1][0:1, 0, 0:1, 0])
            nc.gpsimd.dma_start(out=wao_sb.rearrange(
                "p (m j i c) -> p m j i c", m=8, j=4, i=2, c=128),
                in_=wao_d)
            nc.gpsimd.dma_start(out=w1_sb.rearrange(
                "p (g mi j i c) -> p g mi j i c", g=8, mi=4, j=4, i=2,
                c=128), in_=w1_d)
            nc.gpsimd.dma_start(out=w2_sb.rearrange(
                "p (m j i c) -> p m j i c", m=8, j=16, i=2, c=128),
                in_=w2_d)
            # early barrier: absorb inter-core launch skew under the
            # DMA-bound startup so the real A2A sees aligned peers
            bar_in = dram.tile([NCORES, 4], F8)
            bar_out = dram.tile([NCORES, 4], F8)
            if not single:
                nc.gpsimd.collective_compute(
                    "AllToAll", ALU.bypass,
                    replica_groups=[list(range(NCORES))],
                    ins=[bar_in.opt()], outs=[bar_out.opt()])
            qkv(0)
            vaug_blk(0)
            attn(3)
            norm_half(1)
            qkv(1)
            vaug_blk(1)
            attn(0)
            attn(1)
            norm_half(0)

            if dbg_d is not None:
                nc.sync.dma_start(out=dbg_d[0], in_=qT)
                nc.sync.dma_start(out=dbg_d[1], in_=kT)
                nc.sync.dma_start(out=dbg_d[2], in_=vT)
                for hh in range(2):
                    nc.sync.dma_start(out=dbgf8_d[0][:, N * hh:N * hh + N],
                                      in_=onorm[hh])
                    nc.sync.dma_start(
                        out=dbg32_d[0][:, 512 * hh:512 * hh + 512],
                        in_=den4[hh])
                nc.sync.dma_start(out=dbg32_d[0][:, 1024:1040], in_=rstdc)
                nc.sync.dma_start(out=dbg_d[5][0:4, :], in_=rows4)
                nc.sync.dma_start(out=dbg_d[6][0:1, :], in_=r8row)

            if single:
                nc.sync.dma_start(out=orecvb[:], in_=obounce[:])
            else:
                nc.gpsimd.collective_compute(
                    "AllToAll", ALU.bypass,
                    replica_groups=[list(range(NCORES))],
                    ins=[obounce.opt()], outs=[orecvb.opt()])
            for jp in range(4):
                nc.sync.dma_start(
                    out=orecv2[:, 2 * jp:2 * jp + 2, :],
                    in_=orecvb[2 * jp:2 * jp + 2].rearrange(
                        "k r t -> r k t"))

            ops.release()
            sps.release()
            mmps.release()
            nrmp.release()
            rbp.release()
            ropep.release()
            qtab.release()
            ounp.release()
            xTp.release()

            # ---------------- phase 4: attn_out + residual + LN2 stats -
            x2T = [x2p.tile([128, TOK], FP, name=f"x2T{m}") for m in range(8)]
            x2b = [x2p.tile([128, TOK], BF, name=f"x2b{m}") for m in range(8)]
            sqb = [x2p.tile([128, TOK], BF, name=f"sqb{m}") for m in range(8)]
            h2dr = [x2p.tile([128, 2, TOK], F8, name=f"h2dr{j}")
                    for j in range(4)]
            with tc.tile_pool(name="aops", bufs=3, space="PSUM") as aops, \
                 tc.tile_pool(name="l2ps", bufs=1, space="PSUM") as l2ps, \
                 tc.tile_pool(name="aot", bufs=3) as aot, \
                 tc.tile_pool(name="l2t", bufs=1) as l2t:
                sum_ps = l2ps.tile([1, TOK], FP, tag="l2sum")
                ssq_ps = l2ps.tile([1, TOK], FP, tag="l2ssq")

                def ao_chain(m):
                    ps = aops.tile([128, TOK], FP, tag="aops")
                    for j in range(4):
                        nc.tensor.matmul(
                            ps, waor[:, m, j],
                            orecv2[:, 2 * j:2 * j + 2, :], perf_mode=DR,
                            start=(j == 0), stop=(j == 3),
                            skip_group_check=True)
                    ao_sb = aot.tile([128, TOK], FP, tag="ao")
                    nc.scalar.copy(out=ao_sb, in_=ps)
                    nc.vector.scalar_tensor_tensor(
                        out=x2T[m], in0=ao_sb,
                        scalar=gmsa_sb[:, m:m + 1],
                        in1=xsT[:, m, :], op0=ALU.mult, op1=ALU.add)
                    nc.vector.tensor_copy(out=x2b[m], in_=x2T[m])
                    nc.vector.tensor_mul(sqb[m], x2b[m], x2b[m])

                def ln2_acc(m):
                    nc.tensor.matmul(sum_ps, ones_sb, x2b[m],
                                     start=(m == 0), stop=(m == 7),
                                     skip_group_check=True)
                    nc.tensor.matmul(ssq_ps, ones_sb, sqb[m],
                                     start=(m == 0), stop=(m == 7),
                                     skip_group_check=True)

                ao_chain(0)
                for m in range(1, 8):
                    ao_chain(m)
                    ln2_acc(m - 1)
                ln2_acc(7)

                mu2f = l2t.tile([1, TOK], FP)
                nc.vector.tensor_scalar_mul(mu2f, sum_ps, 1.0 / D)
                var2 = l2t.tile([1, TOK], FP)
                musq = l2t.tile([1, TOK], FP)
                nc.vector.tensor_mul(musq, mu2f, mu2f)
                nc.vector.tensor_scalar_mul(var2, ssq_ps, 1.0 / D)
                nc.vector.tensor_sub(var2, var2, musq)
                sd2 = l2t.tile([1, TOK], FP)
                nc.scalar.activation(out=sd2, in_=var2, func=AF.Sqrt,
                                     bias=eps1, scale=1.0)
                rstd2 = l2t.tile([1, TOK], FP)
                nc.vector.reciprocal_approx_fast(out=rstd2, in_=sd2)
                # row broadcasts (sources live at partition 0)
                mu2bc = l2t.tile([128, TOK], FP)
                nc.gpsimd.partition_broadcast(mu2bc, mu2f)
                rstd2bc = l2t.tile([128, TOK], FP)
                nc.gpsimd.partition_broadcast(rstd2bc, rstd2)
                for k in range(8):
                    u = l2t.tile([128, TOK], FP, tag="u", bufs=2)
                    nc.vector.tensor_sub(u, x2T[k], mu2bc)
                    nc.vector.tensor_mul(h2dr[k // 2][:, k % 2, :],
                                         u, rstd2bc)

            if dbg_d is not None:
                nc.sync.dma_start(
                    out=dbgf8_d[1].rearrange("p (k t) -> p k t", t=TOK),
                    in_=orecv2)
                for m in range(8):
                    nc.sync.dma_start(
                        out=dbg32_d[1][:, TOK * m:TOK * m + TOK],
                        in_=x2T[m])
                for j in range(4):
                    nc.sync.dma_start(
                        out=dbgf8_d[2].rearrange(
                            "p (j i t) -> p j i t", j=4, i=2)[:, j],
                        in_=h2dr[j])

            # ---------------- phase 6: MLP (fp8 DoubleRow) -------------
            g_dr = gp.tile([128, 16, 2, TOK], F8, name="g_dr")
            with tc.tile_pool(name="m1ps", bufs=3, space="PSUM") as m1ps:
                for g in range(8):
                    for mi in range(4):
                        m = 4 * g + mi
                        ps = m1ps.tile([128, TOK], FP, tag="m1")
                        for j in range(4):
                            nc.tensor.matmul(ps, w1r[:, g, mi, j],
                                             h2dr[j], perf_mode=DR,
                                             start=(j == 0), stop=(j == 3))
                        gfunc = (AF.Identity if os.environ.get("DBG_NO_GELU")
                                 else AF.Gelu_apprx_tanh)
                        nc.scalar.activation(out=g_dr[:, m // 2, m % 2, :],
                                             in_=ps, func=gfunc,
                                             bias=b1_sb[:, m:m + 1],
                                             scale=1.0 / W1SCALE)

            outT = gp.tile([128, 8, TOK], FP, name="outT")
            with tc.tile_pool(name="m2ps", bufs=3, space="PSUM") as m2ps, \
                 tc.tile_pool(name="outp", bufs=3) as outp:
                for m in range(8):
                    ps = m2ps.tile([128, TOK], FP, tag="m2")
                    for j in range(16):
                        nc.tensor.matmul(ps, w2r[:, m, j], g_dr[:, j],
                                         perf_mode=DR,
                                         start=(j == 0), stop=(j == 15))
                    # psum = W2SCALE*(m - b2); evac: gmlp/W2SCALE * ps + gb2
                    mo = outp.tile([128, TOK], FP, tag="mo")
                    nc.scalar.activation(out=mo, in_=ps, func=AF.Identity,
                                         bias=gb2_sb[:, m:m + 1],
                                         scale=gmlp_sb[:, m:m + 1])
                    nc.vector.tensor_add(outT[:, m, :], mo, x2T[m])
                    nc.sync.dma_start(out=out_d[:, m, :],
                                      in_=outT[:, m, :])
            w2p.release()
            w1p.release()
            waop.release()

    nc.compile()
    return nc


# ---------------------------------------------------------------------------
# host side
# ---------------------------------------------------------------------------

_NC = None


def _get_nc():
    global _NC
    if _NC is None:
        _NC = build_program()
    return _NC


def _mask01_tiles():
    a = np.arange(128) // BS
    diag = (a[:, None] == a[None, :])
    strict = (a[None, :] > a[:, None])
    incl = (a[None, :] >= a[:, None])
    m = np.concatenate([diag, strict, incl],
                       axis=1).astype(np.float32)
    return np.ascontiguousarray(m.astype(bf16))


def _tile4(wT, km, mm):
    """[K, M] -> (m, p, k, c) with arr[m, p, k, c] = wT[128k+p, 128m+c]."""
    return wT.reshape(km, 128, mm, 128).transpose(2, 1, 0, 3)


def _prep_inputs(x, c, cos, sin, norm1_w, qkv_w, attn_out_w, norm2_w,
                 mlp_w1, mlp_b1, mlp_w2, mlp_b2, adaLN_w, adaLN_b):
    f32 = np.float32
    x = np.asarray(x, f32).reshape(S, D)
    c = np.asarray(c, f32).reshape(COND)
    cos = np.asarray(cos, f32)
    sin = np.asarray(sin, f32)
    qkv_w = np.asarray(qkv_w, f32)
    mlp_w1 = np.asarray(mlp_w1, f32)

    # adaLN modulation on host
    mods = adaLN_w.astype(f32) @ c + np.asarray(adaLN_b, f32)
    sh_msa, sc_msa, g_msa, sh_mlp, sc_mlp, g_mlp = mods.reshape(6, D)

    gam1 = (1.0 + sc_msa) * np.asarray(norm1_w, f32)          # [D]
    qkv_ws = qkv_w * gam1[None, :]                            # [3D, D]
    u_qkv = qkv_ws.sum(axis=1)                                # [3D]
    b_qkv = qkv_w @ sh_msa                                    # [3D]

    gam2 = (1.0 + sc_mlp) * np.asarray(norm2_w, f32)          # [D]
    w1s = mlp_w1 * gam2[None, :]                              # [4D, D]
    b1f = np.asarray(mlp_b1, f32) + mlp_w1 @ sh_mlp           # [4D]
    b2 = np.asarray(mlp_b2, f32)

    xb = x.astype(bf16)
    # LN1 stats on host (fp32)
    mu = x.mean(axis=1)
    sd = np.sqrt(x.var(axis=1) + 1e-5)
    rstd = 1.0 / sd
    rows2 = np.ascontiguousarray(np.stack([-mu, sd]).astype(bf16))
    r8row = np.ascontiguousarray((rstd * 0.125)[None, :].astype(bf16))
    rstdc = np.ascontiguousarray(rstd.reshape(16, 128).T.astype(f32))
    # qkv moving: (n, j, p, i, t) fp8
    xT8 = np.ascontiguousarray(
        x.T.reshape(4, 2, 128, 4, 512).transpose(3, 0, 2, 1, 4)
        .astype(fp8))
    xTb = xb.T.reshape(8, 128, S)                             # (k, p, t)

    # rope tables [2, 128, 1024]: cos | dest-signed sin; 1/WQSCALE folded.
    cs = np.concatenate([cos, cos], axis=-1).T                # [64, N]
    ss = np.concatenate([-sin.T, sin.T], axis=0)              # [64, N]
    cos128 = np.vstack([cs, cs])                              # [128, N]
    sin128 = np.vstack([ss, ss])
    trig = np.ascontiguousarray(
        (np.stack([cos128, sin128]) / WQSCALE).astype(bf16))

    # attn_out: (p=(i,hd), m, j, i2, c); f = (2*(2j+i2)+i)*64 + hd
    waoT = np.ascontiguousarray(
        np.clip(np.asarray(attn_out_w, f32).T * WAOSCALE, -240, 240)
        .reshape(4, 2, 2, 64, 8, 128).transpose(2, 3, 4, 0, 1, 5)
        .reshape(128, 8, 4, 2, 128).astype(fp8))
    # w1: (p, g, mi, j, i, c)
    w1q = np.clip(_tile4(w1s.T, 8, 32) * W1SCALE, -240, 240)  # [32,128,8,128]
    w1T = np.ascontiguousarray(
        w1q.reshape(8, 4, 128, 4, 2, 128).transpose(2, 0, 1, 3, 4, 5)
        .astype(fp8))                                 # (p, g, mi, j, i, c)
    # w2: (p, m, j, i, c)
    w2q = np.clip(_tile4(np.asarray(mlp_w2, f32).T, 32, 8) * W2SCALE,
                  -240, 240)                          # [8, 128, 32, 128]
    w2T = np.ascontiguousarray(
        w2q.reshape(8, 128, 16, 2, 128).transpose(1, 0, 2, 3, 4)
        .astype(fp8))

    smallc = np.ascontiguousarray(np.hstack([
        (g_msa / WAOSCALE).reshape(8, 128).T,
        (g_mlp / W2SCALE).reshape(8, 128).T,
        b1f.reshape(32, 128).T,
        b2.reshape(8, 128).T,
        (g_mlp * b2).reshape(8, 128).T]).astype(f32))         # [128, 64]

    common = {
        "rows2": rows2, "r8row": r8row, "rstdc": rstdc,
        "xT": xT8,
        "waoT": waoT, "w1T": w1T, "w2T": w2T,
        "smallc": smallc, "trig": trig,
        "mask01": _mask01_tiles(),
    }
    in_maps = []
    for j in range(NCORES):
        wq = np.stack([
            np.clip(
                qkv_ws[s * D + 128 * j: s * D + 128 * j + 128].T
                * WQSCALE, -240, 240)
            .reshape(4, 2, 128, 128).transpose(0, 2, 1, 3)
            for s in range(3)])  # [s, j, p, i, c]
        wq = np.ascontiguousarray(
            wq.transpose(2, 0, 1, 3, 4).astype(fp8))  # (p, s, j, i, c)
        ub = np.stack([
            np.concatenate([u_qkv[s * D + 128 * j: s * D + 128 * j + 128]
                            for s in range(3)]),
            np.concatenate([b_qkv[s * D + 128 * j: s * D + 128 * j + 128]
                            for s in range(3)])]) * WQSCALE  # [2, 384]
        m = dict(common)
        m["wqkvT"] = wq
        m["ubrow"] = np.ascontiguousarray(ub.astype(bf16))
        m["xsliceT"] = np.ascontiguousarray(
            xTb[:, :, TOK * j:TOK * j + TOK].transpose(1, 0, 2))  # (p,k,t)
        in_maps.append(m)
    return in_maps


def _assemble(res):
    """Gather per-core outputs [128, 8, TOK] (p, m, t) into [1, S, D]."""
    parts = []
    for j in range(NCORES):
        o = res.results[j]["out"]  # [128, 8, TOK]
        parts.append(np.ascontiguousarray(
            o.transpose(2, 1, 0).reshape(TOK, D)))
    return np.concatenate(parts, axis=0).reshape(1, S, D).astype(np.float32)


def kernel(**inputs):
    nc = _get_nc()
    in_maps = _prep_inputs(**inputs)
    res = run_bass_kernel_spmd(nc, in_maps, core_ids=list(range(NCORES)))
    return _assemble(res)
